# revision 1
# baseline (speedup 1.0000x reference)
"""Trainium2 Bass kernel for DecomposableAttention (B=512, L=256, V=50000, E=300, H=200).

Strategy: data-parallel over batch across 8 cores (64 batches/core).  Per batch:
indirect-DMA gather of embedding rows, on-chip PE transposes to get the
E-on-partitions layout, f32r matmuls for the attend/compare MLPs and the
attention einsums, ACT-exp softmaxes with the length masks folded in as
per-partition -30000 biases, and a final aggregate MLP over all 64 batches.
All matmul free dims are 256 so f32r runs at 1 cycle/row.
"""
import sys

if '/opt/trn_rl_repo' not in sys.path:
    sys.path.insert(0, '/opt/trn_rl_repo')

import numpy as np

B, L, VOCAB, EMBED, HIDDEN = 512, 256, 50000, 300, 200
NCORES = 8
BC = B // NCORES  # batches per core

_prog_cache = {}
USE_F32R = False


def build_program(nb=BC):
    import concourse.bass as bass
    import concourse.bacc as bacc
    import concourse.tile as tile
    import concourse.mybir as mybir
    from concourse.masks import make_identity

    F32 = mybir.dt.float32
    F32R = mybir.dt.float32r if USE_F32R else mybir.dt.float32
    I32 = mybir.dt.int32
    AX = mybir.AxisListType
    ALU = mybir.AluOpType
    ACTF = mybir.ActivationFunctionType
    P = 128
    EK = [(0, 128), (128, 256), (256, 300)]      # E contraction chunks
    H2 = [(0, 100), (100, 200)]                  # H chunks of 100
    E3 = [(0, 100), (100, 200), (200, 300)]      # E output chunks of 100

    nc = bacc.Bacc("TRN2", num_devices=NCORES)

    emb_d = nc.dram_tensor("emb", [VOCAB, EMBED], F32, kind="ExternalInput")
    s1_d = nc.dram_tensor("s1", [nb, L], I32, kind="ExternalInput")
    s2_d = nc.dram_tensor("s2", [nb, L], I32, kind="ExternalInput")
    len1_d = nc.dram_tensor("len1", [nb, 1], I32, kind="ExternalInput")
    len2_d = nc.dram_tensor("len2", [nb, 1], I32, kind="ExternalInput")
    W1a_d = nc.dram_tensor("W1a", [EMBED, HIDDEN], F32R, kind="ExternalInput")
    W2a_d = nc.dram_tensor("W2a", [HIDDEN, HIDDEN], F32R, kind="ExternalInput")
    W1c_d = nc.dram_tensor("W1c", [2 * EMBED, HIDDEN], F32R, kind="ExternalInput")
    W2c_d = nc.dram_tensor("W2c", [HIDDEN, HIDDEN], F32R, kind="ExternalInput")
    W1g_d = nc.dram_tensor("W1g", [2 * HIDDEN, HIDDEN], F32R, kind="ExternalInput")
    W2g_d = nc.dram_tensor("W2g", [HIDDEN, 2], F32R, kind="ExternalInput")
    b1a_d = nc.dram_tensor("b1a", [HIDDEN, 1], F32, kind="ExternalInput")
    b2a_d = nc.dram_tensor("b2a", [HIDDEN, 1], F32, kind="ExternalInput")
    b1c_d = nc.dram_tensor("b1c", [HIDDEN, 1], F32, kind="ExternalInput")
    b2c_d = nc.dram_tensor("b2c", [HIDDEN, 1], F32, kind="ExternalInput")
    b1g_d = nc.dram_tensor("b1g", [HIDDEN, 1], F32, kind="ExternalInput")
    b2g_d = nc.dram_tensor("b2g", [2, 1], F32, kind="ExternalInput")
    out_d = nc.dram_tensor("out", [nb, 2], F32, kind="ExternalOutput")

    with tile.TileContext(nc) as tc:
        import contextlib
        ctx = contextlib.ExitStack()
        with ctx:
            const = ctx.enter_context(tc.tile_pool(name="const", bufs=1))
            psA = ctx.enter_context(tc.tile_pool(name="psA", bufs=3, space="PSUM"))
            psS = ctx.enter_context(tc.tile_pool(name="psS", bufs=3, space="PSUM"))
            psT = ctx.enter_context(tc.tile_pool(name="psT", bufs=2, space="PSUM"))
            gat = ctx.enter_context(tc.tile_pool(name="gat", bufs=3))
            eTp = ctx.enter_context(tc.tile_pool(name="eTp", bufs=2))
            hp = ctx.enter_context(tc.tile_pool(name="hp", bufs=2))
            sm = ctx.enter_context(tc.tile_pool(name="sm", bufs=2))
            att = ctx.enter_context(tc.tile_pool(name="att", bufs=2))
            cmp_ = ctx.enter_context(tc.tile_pool(name="cmp", bufs=2))

            # ---------------- constants ----------------
            ident = const.tile([P, P], F32)
            make_identity(nc, ident[:])
            ident_r = const.tile([P, P], F32R)
            nc.vector.tensor_copy(ident_r[:], ident[:])

            ones_f = const.tile([P, 1], F32)
            nc.vector.memset(ones_f[:], 1.0)
            ones_col_r = const.tile([P, 1], F32R)   # lhsT for den sums (K=128, M=1)
            nc.vector.tensor_copy(ones_col_r[:], ones_f[:])
            ones_row_f = const.tile([1, P], F32)
            nc.vector.memset(ones_row_f[:], 1.0)
            ones_row_r = const.tile([1, P], F32R)   # lhsT for bcasts (K=1, M=128)
            nc.vector.tensor_copy(ones_row_r[:], ones_row_f[:])

            # weights
            W1a_t = [const.tile([k1 - k0, HIDDEN], F32R, name=f"W1a{i}", tag=f"W1a{i}") for i, (k0, k1) in enumerate(EK)]
            for i, (k0, k1) in enumerate(EK):
                nc.sync.dma_start(W1a_t[i][:], W1a_d[k0:k1, :])
            W2a_t = [const.tile([100, HIDDEN], F32R, name=f"W2a{i}", tag=f"W2a{i}") for i in range(2)]
            for i, (k0, k1) in enumerate(H2):
                nc.sync.dma_start(W2a_t[i][:], W2a_d[k0:k1, :])
            W1ca_t = [const.tile([k1 - k0, HIDDEN], F32R, name=f"W1ca{i}", tag=f"W1ca{i}") for i, (k0, k1) in enumerate(EK)]
            for i, (k0, k1) in enumerate(EK):
                nc.sync.dma_start(W1ca_t[i][:], W1c_d[k0:k1, :])
            W1cb_t = [const.tile([100, HIDDEN], F32R, name=f"W1cb{i}", tag=f"W1cb{i}") for i in range(3)]
            for i, (k0, k1) in enumerate(E3):
                nc.sync.dma_start(W1cb_t[i][:], W1c_d[EMBED + k0:EMBED + k1, :])
            W2c_t = [const.tile([100, HIDDEN], F32R, name=f"W2c{i}", tag=f"W2c{i}") for i in range(2)]
            for i, (k0, k1) in enumerate(H2):
                nc.sync.dma_start(W2c_t[i][:], W2c_d[k0:k1, :])
            W1g_t = [const.tile([100, HIDDEN], F32R, name=f"W1g{i}", tag=f"W1g{i}") for i in range(4)]
            for i in range(4):
                nc.sync.dma_start(W1g_t[i][:], W1g_d[i * 100:(i + 1) * 100, :])
            W2g_t = [const.tile([100, 2], F32R, name=f"W2g{i}", tag=f"W2g{i}") for i in range(2)]
            for i, (k0, k1) in enumerate(H2):
                nc.sync.dma_start(W2g_t[i][:], W2g_d[k0:k1, :])

            def bias2(d):
                t = [const.tile([100, 1], F32, name=f"b{d.name}{i}", tag=f"b{d.name}{i}") for i in range(2)]
                for i, (k0, k1) in enumerate(H2):
                    nc.sync.dma_start(t[i][:], d[k0:k1, :])
                return t
            b1a_t, b2a_t = bias2(b1a_d), bias2(b2a_d)
            b1c_t, b2c_t = bias2(b1c_d), bias2(b2c_d)
            b1g_t = bias2(b1g_d)
            b2g_t = const.tile([2, 1], F32)
            nc.sync.dma_start(b2g_t[:], b2g_d[:])

            # masks / lengths
            len1_t = const.tile([nb, 1], I32)
            nc.sync.dma_start(len1_t[:], len1_d[:])
            len2_t = const.tile([nb, 1], I32)
            nc.sync.dma_start(len2_t[:], len2_d[:])
            iota_t = const.tile([nb, L], I32)
            nc.gpsimd.iota(iota_t[:], pattern=[[1, L]], base=0, channel_multiplier=0)

            mask_all = []     # f32 [nb, L] per sentence
            lmT = []          # transposed logmasks: per sentence, 2 tiles [128, nb] f32
            for s, lent in ((0, len1_t), (1, len2_t)):
                m = const.tile([nb, L], F32, name=f"mask{s}", tag=f"mask{s}")
                nc.vector.tensor_tensor(m[:], iota_t[:], lent[:].to_broadcast([nb, L]), op=ALU.is_lt)
                lm = const.tile([nb, L], F32, name=f"lm{s}", tag=f"lm{s}")
                nc.vector.tensor_scalar(lm[:], m[:], 1.0, 30000.0, op0=ALU.subtract, op1=ALU.mult)
                lts = []
                for c in range(2):
                    tp = psT.tile([P, nb], F32, name="lmT_ps", tag="tr")
                    nc.tensor.transpose(tp[:], lm[:, c * P:(c + 1) * P], ident[:nb, :nb])
                    lt = const.tile([P, nb], F32, name=f"lmT{s}{c}", tag=f"lmT{s}{c}")
                    nc.vector.tensor_copy(lt[:], tp[:])
                    lts.append(lt)
                mask_all.append(m)
                lmT.append(lts)

            # per-batch masks are built as tiny [1, L] rows from iota_row + len_f
            len_f = []
            for s, ld in ((0, len1_d), (1, len2_d)):
                lf = const.tile([1, nb], I32, name=f"lenf{s}", tag=f"lenf{s}")
                nc.sync.dma_start(lf[:], ld[:].rearrange("n one -> one n"))
                len_f.append(lf)
            iota_row = const.tile([1, L], I32)
            nc.gpsimd.iota(iota_row[:], pattern=[[1, L]], base=0, channel_multiplier=0)

            # token indices, transposed to [128, nb] int32 per chunk
            sT = []
            for s, sd in ((0, s1_d), (1, s2_d)):
                st = const.tile([nb, L], I32, name=f"s{s}", tag=f"s{s}")
                nc.sync.dma_start(st[:], sd[:])
                sf = const.tile([nb, L], F32, name=f"sf{s}", tag=f"sf{s}")
                nc.vector.tensor_copy(sf[:], st[:])
                chunks = []
                for c in range(2):
                    tp = psT.tile([P, nb], F32, name="sT_ps", tag="tr")
                    nc.tensor.transpose(tp[:], sf[:, c * P:(c + 1) * P], ident[:nb, :nb])
                    tf = const.tile([P, nb], F32, name=f"sTf{s}{c}", tag=f"sTf{s}{c}")
                    nc.vector.tensor_copy(tf[:], tp[:])
                    ti = const.tile([P, nb], I32, name=f"sTi{s}{c}", tag=f"sTi{s}{c}")
                    nc.vector.tensor_copy(ti[:], tf[:])
                    chunks.append(ti)
                sT.append(chunks)

            # v accumulators [100, nb] per H-chunk per sentence
            v_all = [[const.tile([100, nb], F32, name=f"v{s}{m}", tag=f"v{s}{m}") for m in range(2)] for s in range(2)]

            # ---------------- per-batch loop ----------------
            for b in range(nb):
                # mask rows [1, L] for this batch (partition 0)
                mrow = []    # f32
                mrow_r = []  # f32r
                for s in range(2):
                    mr = sm.tile([1, L], F32, name=f"mrow{s}", tag=f"mrow{s}")
                    nc.vector.tensor_tensor(mr[:], iota_row[:],
                                            len_f[s][:, b:b + 1].to_broadcast([1, L]), op=ALU.is_lt)
                    mrr = sm.tile([1, L], F32R, name=f"mrowr{s}", tag=f"mrowr{s}")
                    nc.vector.tensor_copy(mrr[:], mr[:])
                    mrow.append(mr)
                    mrow_r.append(mrr)
                eR = [[], []]   # f32r natural [128, 300] x2 chunks per sentence
                eT = [[], []]   # f32r transposed [(128|128|44), 256] x3 per sentence
                hT = [[], []]   # f32r [100, 256] x2 per sentence
                for s in range(2):
                    for c in range(2):
                        en = gat.tile([P, EMBED], F32, name=f"eN{s}{c}", tag=f"eN{s}{c}")
                        nc.gpsimd.indirect_dma_start(
                            out=en[:], out_offset=None, in_=emb_d[:],
                            in_offset=bass.IndirectOffsetOnAxis(ap=sT[s][c][:, b:b + 1], axis=0),
                        )
                        if USE_F32R:
                            er = gat.tile([P, EMBED], F32R, name=f"eR{s}{c}", tag=f"eR{s}{c}")
                            nc.vector.tensor_copy(er[:], en[:])
                        else:
                            er = en
                        eR[s].append(er)
                    for k, (k0, k1) in enumerate(EK):
                        et = eTp.tile([k1 - k0, L], F32R, name=f"eT{s}{k}", tag=f"eT{s}{k}")
                        for c in range(2):
                            tp = psT.tile([P, P], F32R, name="tr_ps", tag="tr")
                            nc.tensor.transpose(tp[:k1 - k0, :], eR[s][c][:, k0:k1], ident_r[:])
                            nc.any.tensor_copy(et[:, c * P:(c + 1) * P], tp[:k1 - k0, :])
                        eT[s].append(et)
                # attend MLP for both sentences, interleaved so each weight
                # chunk's LDWEIGHTS is reused by the second sentence's matmul
                ha = [[], []]
                for m, (m0, m1) in enumerate(H2):
                    pp = [psA.tile([100, L], F32, name=f"h1_ps{s}", tag="mm") for s in range(2)]
                    for k in range(3):
                        for s in range(2):
                            nc.tensor.matmul(pp[s][:], W1a_t[k][:, m0:m1], eT[s][k][:],
                                             start=(k == 0), stop=(k == 2))
                    for s in range(2):
                        h = hp.tile([100, L], F32R, name=f"ha{s}{m}", tag=f"ha{s}{m}")
                        nc.scalar.activation(h[:], pp[s][:], ACTF.Relu, bias=b1a_t[m][:], scale=1.0)
                        ha[s].append(h)
                for m, (m0, m1) in enumerate(H2):
                    qp = [psA.tile([100, L], F32, name=f"h2_ps{s}", tag="mm") for s in range(2)]
                    for k2 in range(2):
                        for s in range(2):
                            nc.tensor.matmul(qp[s][:], W2a_t[k2][:, m0:m1], ha[s][k2][:],
                                             start=(k2 == 0), stop=(k2 == 1))
                    for s in range(2):
                        h = hp.tile([100, L], F32R, name=f"hT{s}{m}", tag=f"hT{s}{m}")
                        nc.scalar.activation(h[:], qp[s][:], ACTF.Relu, bias=b2a_t[m][:], scale=1.0)
                        hT[s].append(h)

                # scores: e [i, j] and e^T [j, i]; copy out of PSUM immediately
                # (frees score banks for the next batch's matmuls)
                e_sb, eT_sb = [], []
                for ic in range(2):
                    ep = psS.tile([P, L], F32, name=f"e_ps{ic}", tag="score")
                    for m in range(2):
                        nc.tensor.matmul(ep[:], hT[0][m][:, ic * P:(ic + 1) * P], hT[1][m][:],
                                         start=(m == 0), stop=(m == 1))
                    es = sm.tile([P, L], F32, name=f"e_sb{ic}", tag=f"e_sb{ic}")
                    nc.scalar.activation(es[:], ep[:], ACTF.Identity, bias=0.0, scale=1.0)
                    e_sb.append(es)
                for jc in range(2):
                    ep = psS.tile([P, L], F32, name=f"eT_ps{jc}", tag="score")
                    for m in range(2):
                        nc.tensor.matmul(ep[:], hT[1][m][:, jc * P:(jc + 1) * P], hT[0][m][:],
                                         start=(m == 0), stop=(m == 1))
                    es = sm.tile([P, L], F32, name=f"eT_sb{jc}", tag=f"eT_sb{jc}")
                    nc.scalar.activation(es[:], ep[:], ACTF.Identity, bias=0.0, scale=1.0)
                    eT_sb.append(es)

                # M = rowmax(e); broadcast tile Mb [128, 256]
                Mt_ps = psA.tile([1, L], F32, name="Mt_ps", tag="mm")
                for ic in range(2):
                    mp = sm.tile([P, 1], F32, name=f"M_p{ic}", tag=f"M_p{ic}")
                    nc.vector.tensor_reduce(mp[:], e_sb[ic][:], axis=AX.X, op=ALU.max)
                    nc.tensor.transpose(Mt_ps[:, ic * P:(ic + 1) * P], mp[:], ident[:])
                M_r = sm.tile([1, L], F32R, name="M_r", tag="M_r")
                nc.vector.tensor_copy(M_r[:], Mt_ps[:])
                Mb_ps = psA.tile([P, L], F32, name="Mb_ps", tag="mm")
                nc.tensor.matmul(Mb_ps[:], ones_row_r[:], M_r[:], start=True, stop=True)
                Mb = sm.tile([P, L], F32, name="Mb", tag="Mb")
                nc.any.tensor_copy(Mb[:], Mb_ps[:])

                # exp(e - M[j]) * mask1[i];  exp(eT - M[i]) * mask2[j]
                u = [[], []]  # u[0]=uA (i-part), u[1]=uB (j-part)
                for d, (eps, lmTs) in enumerate(((e_sb, lmT[0]), (eT_sb, lmT[1]))):
                    for c in range(2):
                        t = sm.tile([P, L], F32, name=f"t{d}{c}", tag=f"t{d}{c}")
                        nc.vector.tensor_tensor(t[:], eps[c][:], Mb[:], op=ALU.subtract)
                        uu = sm.tile([P, L], F32R, name=f"u{d}{c}", tag=f"u{d}{c}")
                        nc.scalar.activation(uu[:], t[:], ACTF.Exp, bias=lmTs[c][:, b:b + 1], scale=1.0)
                        u[d].append(uu)

                # denominators and normalizer broadcast tiles
                R_bc = []
                for d in range(2):
                    dp = psA.tile([1, L], F32, name=f"den_ps{d}", tag="mm")
                    for c in range(2):
                        nc.tensor.matmul(dp[:], ones_col_r[:], u[d][c][:],
                                         start=(c == 0), stop=(c == 1))
                    rc = sm.tile([1, L], F32, name=f"recip{d}", tag=f"recip{d}")
                    nc.vector.reciprocal(rc[:], dp[:])
                    rm = sm.tile([1, L], F32R, name=f"recipm{d}", tag=f"recipm{d}")
                    # direction A (d=0) masks output cols by mask2; B by mask1
                    nc.vector.tensor_tensor(rm[:], rc[:], mrow[1 - d][:], op=ALU.mult)
                    rp = psA.tile([P, L], F32, name=f"R_ps{d}", tag="mm")
                    nc.tensor.matmul(rp[:], ones_row_r[:], rm[:], start=True, stop=True)
                    rb = sm.tile([P, L], F32, name=f"R_bc{d}", tag=f"R_bc{d}")
                    nc.any.tensor_copy(rb[:], rp[:])
                    R_bc.append(rb)

                # attention sums: alphas^T = e1^T-chunks @ uA, betas^T = e2-chunks @ uB
                xT = [[], []]   # xT[0]=alphasT (for sentence-2 compare), xT[1]=betasT
                for d in range(2):
                    for m3, (m0, m1) in enumerate(E3):
                        ap_ = psA.tile([100, L], F32, name="attn_ps", tag="mm")
                        for c in range(2):
                            nc.tensor.matmul(ap_[:], eR[d][c][:, m0:m1], u[d][c][:],
                                             start=(c == 0), stop=(c == 1))
                        at = att.tile([100, L], F32R, name=f"xT{d}{m3}", tag=f"xT{d}{m3}")
                        nc.vector.tensor_tensor(at[:], ap_[:], R_bc[d][:100, :], op=ALU.mult)
                        xT[d].append(at)

                # compare MLP + masked sum.  sentence 1 pairs with betasT (xT[1]), mask1;
                # sentence 2 pairs with alphasT (xT[0]), mask2.
                # compare MLP, sentences interleaved per weight chunk.
                # sentence 0 pairs eT[0] with betasT (xT[1]); sentence 1 pairs
                # eT[1] with alphasT (xT[0]).
                r1 = [[], []]
                for m, (m0, m1) in enumerate(H2):
                    up = [psA.tile([100, L], F32, name=f"c1_ps{s}", tag="mm") for s in range(2)]
                    for k in range(3):
                        for s in range(2):
                            nc.tensor.matmul(up[s][:], W1ca_t[k][:, m0:m1], eT[s][k][:],
                                             start=(k == 0), stop=False)
                    for k3 in range(3):
                        for s in range(2):
                            nc.tensor.matmul(up[s][:], W1cb_t[k3][:, m0:m1], xT[1 - s][k3][:],
                                             start=False, stop=(k3 == 2))
                    for s in range(2):
                        r = cmp_.tile([100, L], F32R, name=f"r1{s}{m}", tag=f"r1{s}{m}")
                        nc.scalar.activation(r[:], up[s][:], ACTF.Relu, bias=b1c_t[m][:], scale=1.0)
                        r1[s].append(r)
                maskbc = []
                for s in range(2):
                    mb_ps = psA.tile([P, L], F32, name=f"maskbc_ps{s}", tag="mm")
                    nc.tensor.matmul(mb_ps[:], ones_row_r[:], mrow_r[s][:],
                                     start=True, stop=True)
                    mb_sb = cmp_.tile([100, L], F32, name=f"maskbc{s}", tag=f"maskbc{s}")
                    nc.any.tensor_copy(mb_sb[:], mb_ps[:100, :])
                    maskbc.append(mb_sb)
                for m, (m0, m1) in enumerate(H2):
                    cp = [psA.tile([100, L], F32, name=f"c2_ps{s}", tag="mm") for s in range(2)]
                    for k2 in range(2):
                        for s in range(2):
                            nc.tensor.matmul(cp[s][:], W2c_t[k2][:, m0:m1], r1[s][k2][:],
                                             start=(k2 == 0), stop=(k2 == 1))
                    for s in range(2):
                        c2 = cmp_.tile([100, L], F32, name=f"c2{s}{m}", tag=f"c2{s}{m}")
                        nc.scalar.activation(c2[:], cp[s][:], ACTF.Relu, bias=b2c_t[m][:], scale=1.0)
                        scr = cmp_.tile([100, L], F32, name=f"scr{s}{m}", tag=f"scr{s}{m}")
                        nc.vector.tensor_tensor(scr[:], c2[:], maskbc[s][:], op=ALU.mult)
                        nc.vector.tensor_reduce(v_all[s][m][:, b:b + 1], scr[:], axis=AX.X, op=ALU.add)

            # ---------------- aggregate ----------------
            vr = []
            for s in range(2):
                for m in range(2):
                    t = const.tile([100, nb], F32R, name=f"vr{s}{m}", tag=f"vr{s}{m}")
                    nc.vector.tensor_copy(t[:], v_all[s][m][:])
                    vr.append(t)
            g1 = []
            for m, (m0, m1) in enumerate(H2):
                gp = psA.tile([100, nb], F32, name="g_ps", tag="mm")
                for k in range(4):
                    nc.tensor.matmul(gp[:], W1g_t[k][:, m0:m1], vr[k][:],
                                     start=(k == 0), stop=(k == 3))
                g = const.tile([100, nb], F32R, name=f"g1{m}", tag=f"g1{m}")
                nc.scalar.activation(g[:], gp[:], ACTF.Relu, bias=b1g_t[m][:], scale=1.0)
                g1.append(g)
            op = psA.tile([2, nb], F32, name="o_ps", tag="mm")
            for k2 in range(2):
                nc.tensor.matmul(op[:], W2g_t[k2][:], g1[k2][:],
                                 start=(k2 == 0), stop=(k2 == 1))
            osb = const.tile([2, nb], F32, name="osb", tag="osb")
            nc.scalar.activation(osb[:], op[:], ACTF.Identity, bias=b2g_t[:], scale=1.0)
            nc.sync.dma_start(out_d[:].rearrange("b o -> o b"), osb[:])

    nc.compile()
    return nc


def _shard_inputs(inputs, nb=BC, ncores=NCORES):
    f = np.ascontiguousarray
    maps = []
    for c in range(ncores):
        sl = slice(c * nb, (c + 1) * nb)
        maps.append(dict(
            emb=f(inputs['emb'].astype(np.float32)),
            s1=f(inputs['s1'][sl].astype(np.int32)),
            s2=f(inputs['s2'][sl].astype(np.int32)),
            len1=f(inputs['len1'][sl].reshape(nb, 1).astype(np.int32)),
            len2=f(inputs['len2'][sl].reshape(nb, 1).astype(np.int32)),
            W1a=f(inputs['W1a'].astype(np.float32)),
            W2a=f(inputs['W2a'].astype(np.float32)),
            W1c=f(inputs['W1c'].astype(np.float32)),
            W2c=f(inputs['W2c'].astype(np.float32)),
            W1g=f(inputs['W1g'].astype(np.float32)),
            W2g=f(inputs['W2g'].astype(np.float32)),
            b1a=f(inputs['b1a'].reshape(-1, 1).astype(np.float32)),
            b2a=f(inputs['b2a'].reshape(-1, 1).astype(np.float32)),
            b1c=f(inputs['b1c'].reshape(-1, 1).astype(np.float32)),
            b2c=f(inputs['b2c'].reshape(-1, 1).astype(np.float32)),
            b1g=f(inputs['b1g'].reshape(-1, 1).astype(np.float32)),
            b2g=f(inputs['b2g'].reshape(-1, 1).astype(np.float32)),
        ))
    return maps


def kernel(**inputs):
    from concourse.bass_utils import run_bass_kernel_spmd
    if 'prog' not in _prog_cache:
        _prog_cache['prog'] = build_program(BC)
    nc = _prog_cache['prog']
    in_maps = _shard_inputs(inputs)
    res = run_bass_kernel_spmd(nc, in_maps, core_ids=list(range(NCORES)))
    out = np.concatenate([res.results[c]["out"] for c in range(NCORES)], axis=0)
    return out.astype(np.float32)



# revision 17
# speedup vs baseline: 1.5491x; 1.5491x over previous
"""Trainium2 Bass kernel for DecomposableAttention (B=512, L=256, V=50000, E=300, H=200).

Data-parallel over batch across 8 cores (64 batches/core).  All matmuls in
bf16 (1 cycle/row on the PE), fp32 PSUM accumulation.  Per batch: indirect-DMA
gather of bf16 embedding rows (table converted host-side), PE transposes to the
E-on-partitions layout, attend MLP, scores in one direction only (the exp'd
scores are PE-transposed for the reverse direction), softmax without the max
stabilizer (scores are bounded ~11 so exp never overflows; the stabilizer
cancels exactly after normalization), attention sums chunked to line up with
the compare MLP's 5 packed 128-row K-chunks, denominators via M=1 matmuls with
ones/mask columns as the stationary operand, and a fused masked-sum via a K=1
logmask matmul into PSUM + the activation engine's accum_out.  PSUM is only 8
banks of 2 KB; pairs of [*,256] f32 outputs share one [128,512] bank tile.
"""
import sys

if '/opt/trn_rl_repo' not in sys.path:
    sys.path.insert(0, '/opt/trn_rl_repo')

import numpy as np

B, L, VOCAB, EMBED, HIDDEN = 512, 256, 50000, 300, 200
NCORES = 8
BC = B // NCORES  # batches per core
P = 128

_prog_cache = {}

# K-chunk layout of the 600-dim compare input [e (300); beta (300)].
# Chunk 2 mixes the e tail (44 rows) + 20 zero rows + beta[0:64] at partition
# offset 64 (PE outputs with M<=64 may target offset 64); W1c is host-packed
# to match.
XCHUNK = [128, 128, 128, 128, 108]
ACHUNK = [(0, 64, 64), (64, 192, 0), (192, 300, 0)]  # attn-out col ranges + psum offset


def build_program(nb=BC, ndev=NCORES, dbg=False):
    import concourse.bass as bass
    import concourse.bacc as bacc
    import concourse.tile as tile
    import concourse.mybir as mybir
    from concourse.masks import make_identity

    F32 = mybir.dt.float32
    BF = mybir.dt.bfloat16
    I32 = mybir.dt.int32
    ALU = mybir.AluOpType
    ACTF = mybir.ActivationFunctionType
    EK = [(0, 128), (128, 256), (256, 300)]   # E contraction chunks
    HM = [(0, 128), (128, 200)]               # H chunks (128, 72)

    nc = bacc.Bacc("TRN2", num_devices=ndev)

    emb_d = nc.dram_tensor("emb", [VOCAB, EMBED], BF, kind="ExternalInput")
    s1_d = nc.dram_tensor("s1", [nb, L], I32, kind="ExternalInput")
    s2_d = nc.dram_tensor("s2", [nb, L], I32, kind="ExternalInput")
    len1_d = nc.dram_tensor("len1", [nb, 1], I32, kind="ExternalInput")
    len2_d = nc.dram_tensor("len2", [nb, 1], I32, kind="ExternalInput")
    W1a_d = nc.dram_tensor("W1a", [EMBED, HIDDEN], BF, kind="ExternalInput")
    W2a_d = nc.dram_tensor("W2a", [HIDDEN, HIDDEN], BF, kind="ExternalInput")
    W1c_d = nc.dram_tensor("W1c", [640, HIDDEN], BF, kind="ExternalInput")   # host-packed 5x128
    W2c_d = nc.dram_tensor("W2c", [HIDDEN, HIDDEN], BF, kind="ExternalInput")
    W1g_d = nc.dram_tensor("W1g", [2 * HIDDEN, HIDDEN], BF, kind="ExternalInput")
    W2g_d = nc.dram_tensor("W2g", [HIDDEN, 2], BF, kind="ExternalInput")
    b1a_d = nc.dram_tensor("b1a", [HIDDEN, 1], F32, kind="ExternalInput")
    b2a_d = nc.dram_tensor("b2a", [HIDDEN, 1], F32, kind="ExternalInput")
    b1c_d = nc.dram_tensor("b1c", [HIDDEN, 1], F32, kind="ExternalInput")
    b2c_d = nc.dram_tensor("b2c", [HIDDEN, 1], F32, kind="ExternalInput")
    b1g_d = nc.dram_tensor("b1g", [HIDDEN, 1], F32, kind="ExternalInput")
    b2g_d = nc.dram_tensor("b2g", [2, 1], F32, kind="ExternalInput")
    out_d = nc.dram_tensor("out", [nb, 2], F32, kind="ExternalOutput")
    if dbg:
        dbg_d = {
            'xt0': nc.dram_tensor("dbg_xt0", [P, 5 * L], BF, kind="ExternalOutput"),
            'xt1': nc.dram_tensor("dbg_xt1", [P, 5 * L], BF, kind="ExternalOutput"),
            'u0': nc.dram_tensor("dbg_u0", [P, 2 * L], BF, kind="ExternalOutput"),
            'u1': nc.dram_tensor("dbg_u1", [P, 2 * L], BF, kind="ExternalOutput"),
            'hT00': nc.dram_tensor("dbg_hT00", [P, L], BF, kind="ExternalOutput"),
            'de0': nc.dram_tensor("dbg_de0", [1, L], F32, kind="ExternalOutput"),
            'de1': nc.dram_tensor("dbg_de1", [1, L], F32, kind="ExternalOutput"),
            'rm0': nc.dram_tensor("dbg_rm0", [1, L], BF, kind="ExternalOutput"),
            'rm1': nc.dram_tensor("dbg_rm1", [1, L], BF, kind="ExternalOutput"),
            'v00': nc.dram_tensor("dbg_v00", [P, nb], F32, kind="ExternalOutput"),
            'eR00': nc.dram_tensor("dbg_eR00", [P, EMBED], BF, kind="ExternalOutput"),
        }

    with tile.TileContext(nc) as tc:
        import contextlib
        ctx = contextlib.ExitStack()
        with ctx:
            const = ctx.enter_context(tc.tile_pool(name="const", bufs=1))
            # PSUM: 8 banks of 2KB.  psT: [128,768] bf16 (eT triple / u1 pair)
            # bufs=2 -> 2 banks.  psMM: [128,512] f32 pairs bufs=2 -> 2 banks.
            # psS: [128,512] f32 bufs=1 -> 1 bank (scores pair, R pair, setup
            # transposes).  psA: [128,512] f32 bufs=2 -> 2 banks.  Total 7.
            psT = ctx.enter_context(tc.tile_pool(name="psT", bufs=2, space="PSUM"))
            psMM = ctx.enter_context(tc.tile_pool(name="psMM", bufs=2, space="PSUM"))
            psS = ctx.enter_context(tc.tile_pool(name="psS", bufs=1, space="PSUM"))
            psA = ctx.enter_context(tc.tile_pool(name="psA", bufs=2, space="PSUM"))
            gat = ctx.enter_context(tc.tile_pool(name="gat", bufs=2))
            xtp = ctx.enter_context(tc.tile_pool(name="xtp", bufs=2))
            hp = ctx.enter_context(tc.tile_pool(name="hp", bufs=2))
            sm = ctx.enter_context(tc.tile_pool(name="sm", bufs=2))

            def ps_pair(pool, nm):
                return pool.tile([P, 512], F32, name=nm, tag=pool.name)

            def ps_tr(nm):
                # padded to 2KB so each pool buffer is PSUM-bank aligned
                return psT.tile([P, 1024], BF, name=nm, tag="psT")

            # ---------------- constants ----------------
            ident = const.tile([P, P], F32)
            make_identity(nc, ident[:])
            ident_b = const.tile([P, P], BF)
            nc.vector.tensor_copy(ident_b[:], ident[:])
            ones_row_b = const.tile([1, P], BF)
            nc.vector.memset(ones_row_b[:], 1.0)
            ones_col_b = const.tile([P, 1], BF)
            nc.vector.memset(ones_col_b[:], 1.0)

            # weights (bf16, direct DMA)
            W1a_t = [const.tile([k1 - k0, HIDDEN], BF, name=f"W1a{i}", tag=f"W1a{i}")
                     for i, (k0, k1) in enumerate(EK)]
            for i, (k0, k1) in enumerate(EK):
                nc.sync.dma_start(W1a_t[i][:], W1a_d[k0:k1, :])
            W2a_t = [const.tile([m1 - m0, HIDDEN], BF, name=f"W2a{i}", tag=f"W2a{i}")
                     for i, (m0, m1) in enumerate(HM)]
            for i, (m0, m1) in enumerate(HM):
                nc.sync.dma_start(W2a_t[i][:], W2a_d[m0:m1, :])
            W1c_t = [const.tile([k, HIDDEN], BF, name=f"W1c{i}", tag=f"W1c{i}")
                     for i, k in enumerate(XCHUNK)]
            for i in range(5):
                nc.sync.dma_start(W1c_t[i][:], W1c_d[i * 128:i * 128 + XCHUNK[i], :])
            W2c_t = [const.tile([m1 - m0, HIDDEN], BF, name=f"W2c{i}", tag=f"W2c{i}")
                     for i, (m0, m1) in enumerate(HM)]
            for i, (m0, m1) in enumerate(HM):
                nc.sync.dma_start(W2c_t[i][:], W2c_d[m0:m1, :])
            GK = [(0, 128), (128, 200), (200, 328), (328, 400)]
            W1g_t = [const.tile([k1 - k0, HIDDEN], BF, name=f"W1g{i}", tag=f"W1g{i}")
                     for i, (k0, k1) in enumerate(GK)]
            for i, (k0, k1) in enumerate(GK):
                nc.sync.dma_start(W1g_t[i][:], W1g_d[k0:k1, :])
            W2g_t = [const.tile([m1 - m0, 2], BF, name=f"W2g{i}", tag=f"W2g{i}")
                     for i, (m0, m1) in enumerate(HM)]
            for i, (m0, m1) in enumerate(HM):
                nc.sync.dma_start(W2g_t[i][:], W2g_d[m0:m1, :])

            def bias2(d, nm):
                t = [const.tile([m1 - m0, 1], F32, name=f"b{nm}{i}", tag=f"b{nm}{i}")
                     for i, (m0, m1) in enumerate(HM)]
                for i, (m0, m1) in enumerate(HM):
                    nc.sync.dma_start(t[i][:], d[m0:m1, :])
                return t
            b1a_t, b2a_t = bias2(b1a_d, "1a"), bias2(b2a_d, "2a")
            b1c_t, b2c_t = bias2(b1c_d, "1c"), bias2(b2c_d, "2c")
            b1g_t = bias2(b1g_d, "1g")
            b2g_t = const.tile([2, 1], F32)
            nc.sync.dma_start(b2g_t[:], b2g_d[:])

            # masks / lengths
            len_t = []
            len_f = []
            for s, ld in ((0, len1_d), (1, len2_d)):
                lt = const.tile([nb, 1], I32, name=f"len{s}", tag=f"len{s}")
                nc.sync.dma_start(lt[:], ld[:])
                len_t.append(lt)
                lf = const.tile([1, nb], I32, name=f"lenf{s}", tag=f"lenf{s}")
                nc.sync.dma_start(lf[:], ld[:].rearrange("n one -> one n"))
                len_f.append(lf)
            iota_t = const.tile([nb, L], I32)
            nc.gpsimd.iota(iota_t[:], pattern=[[1, L]], base=0, channel_multiplier=0)
            iota_row = const.tile([1, L], I32)
            nc.gpsimd.iota(iota_row[:], pattern=[[1, L]], base=0, channel_multiplier=0)

            lmT = []   # transposed logmask [128, nb] f32, 2 chunks (exp bias)
            mTb = []   # transposed 0/1 mask [128, nb] bf16, 2 chunks (den lhsT)
            for s in range(2):
                m = const.tile([nb, L], F32, name=f"mask{s}", tag=f"mask{s}")
                nc.vector.tensor_tensor(m[:], iota_t[:], len_t[s][:].to_broadcast([nb, L]), op=ALU.is_lt)
                lm = const.tile([nb, L], F32, name=f"lm{s}", tag=f"lm{s}")
                nc.vector.tensor_scalar(lm[:], m[:], 1.0, 30000.0, op0=ALU.subtract, op1=ALU.mult)
                lts, mts = [], []
                for c in range(2):
                    if s == 0:  # only logmask1 is used (exp bias)
                        tp = ps_pair(psS, "setup_tp")
                        nc.tensor.transpose(tp[:, 0:nb], lm[:, c * P:(c + 1) * P], ident[:nb, :nb])
                        lt = const.tile([P, nb], F32, name=f"lmT{s}{c}", tag=f"lmT{s}{c}")
                        nc.vector.tensor_copy(lt[:], tp[:, 0:nb])
                        lts.append(lt)
                    if s == 1:  # only mask2 cols are used (e2m, den B)
                        tp2 = ps_pair(psS, "setup_tp")
                        nc.tensor.transpose(tp2[:, 0:nb], m[:, c * P:(c + 1) * P], ident[:nb, :nb])
                        mt = const.tile([P, nb], BF, name=f"mTb{s}{c}", tag=f"mTb{s}{c}")
                        nc.vector.tensor_copy(mt[:], tp2[:, 0:nb])
                        mts.append(mt)
                lmT.append(lts)
                mTb.append(mts)

            # token indices, transposed to [128, nb] int32 per chunk
            sT = []
            for s, sd in ((0, s1_d), (1, s2_d)):
                st = const.tile([nb, L], I32, name=f"s{s}", tag=f"s{s}")
                nc.sync.dma_start(st[:], sd[:])
                sf = const.tile([nb, L], F32, name=f"sf{s}", tag=f"sf{s}")
                nc.vector.tensor_copy(sf[:], st[:])
                chunks = []
                for c in range(2):
                    tp = ps_pair(psS, "setup_tp")
                    nc.tensor.transpose(tp[:, 0:nb], sf[:, c * P:(c + 1) * P], ident[:nb, :nb])
                    tf = const.tile([P, nb], F32, name=f"sTf{s}{c}", tag=f"sTf{s}{c}")
                    nc.vector.tensor_copy(tf[:], tp[:, 0:nb])
                    ti = const.tile([P, nb], I32, name=f"sTi{s}{c}", tag=f"sTi{s}{c}")
                    nc.vector.tensor_copy(ti[:], tf[:])
                    chunks.append(ti)
                sT.append(chunks)

            # v accumulators [(128|72), nb] per H-chunk per sentence
            v_all = [[const.tile([m1 - m0, nb], F32, name=f"v{s}{m}", tag=f"v{s}{m}")
                      for m, (m0, m1) in enumerate(HM)] for s in range(2)]

            # ---------------- per-batch loop ----------------
            for b in range(nb):
                # per-batch mask rows on partition 0
                mrow, lmrow = [], []
                for s in range(2):
                    mr = sm.tile([1, L], F32, name=f"mrow{s}", tag=f"mrow{s}")
                    nc.vector.tensor_tensor(mr[:], iota_row[:],
                                            len_f[s][:, b:b + 1].to_broadcast([1, L]), op=ALU.is_lt)
                    lr = sm.tile([1, L], BF, name=f"lmrow{s}", tag=f"lmrow{s}")
                    nc.vector.tensor_scalar(lr[:], mr[:], 1.0, 30000.0,
                                            op0=ALU.subtract, op1=ALU.mult)
                    mrow.append(mr)
                    lmrow.append(lr)

                # gather embeddings (bf16 rows)
                eR = [[], []]
                for s in range(2):
                    for c in range(2):
                        er = gat.tile([P, EMBED], BF, name=f"eR{s}{c}", tag=f"eR{s}{c}")
                        nc.gpsimd.indirect_dma_start(
                            out=er[:], out_offset=None, in_=emb_d[:],
                            in_offset=bass.IndirectOffsetOnAxis(ap=sT[s][c][:, b:b + 1], axis=0),
                        )
                        eR[s].append(er)
                # e2m = e2R * mask2 (per-partition)
                e2m = []
                for c in range(2):
                    em = gat.tile([P, EMBED], BF, name=f"e2m{c}", tag=f"e2m{c}")
                    nc.vector.tensor_tensor(em[:], eR[1][c][:],
                                            mTb[1][c][:, b:b + 1].to_broadcast([P, EMBED]), op=ALU.mult)
                    e2m.append(em)

                # x input [128, 5*256] bf16 per sentence: chunk k at free
                # [k*256:(k+1)*256], K=XCHUNK[k] partitions.  Chunks 0,1 = eT
                # rows 0:256; chunk2 = eT 256:300 | 20 zero rows | beta 0:64 at
                # partition 64; chunk3 = beta 64:192; chunk4 = beta 192:300.
                xt = [xtp.tile([P, 5 * L], BF, name=f"xt{s}", tag=f"xt{s}") for s in range(2)]
                for s in range(2):
                    # rows 44:64 of chunk 2 are zero padding; 0:44 are
                    # overwritten by the eT copy below (starts must be
                    # 32-partition aligned, so clear 0:64)
                    nc.vector.memset(xt[s][0:64, 2 * L:3 * L], 0.0)
                    tp = ps_tr(f"eTtp{s}")
                    for k, (k0, k1) in enumerate(EK):
                        for c in range(2):
                            nc.tensor.transpose(tp[:k1 - k0, k * L + c * P:k * L + (c + 1) * P],
                                                eR[s][c][:, k0:k1], ident_b[:])
                    nc.any.tensor_copy(xt[s][:, 0:2 * L], tp[:, 0:2 * L])
                    nc.any.tensor_copy(xt[s][0:44, 2 * L:3 * L], tp[0:44, 2 * L:3 * L])

                # attend MLP
                ha = [[], []]
                for m, (m0, m1) in enumerate(HM):
                    pa = ps_pair(psMM, "pa")
                    for s in range(2):
                        for k, (k0, k1) in enumerate(EK):
                            nc.tensor.matmul(pa[:m1 - m0, s * L:(s + 1) * L],
                                             W1a_t[k][:, m0:m1], xt[s][0:k1 - k0, k * L:(k + 1) * L],
                                             start=(k == 0), stop=(k == 2))
                    for s in range(2):
                        h = hp.tile([m1 - m0, L], BF, name=f"ha{s}{m}", tag=f"ha{s}{m}")
                        nc.scalar.activation(h[:], pa[:m1 - m0, s * L:(s + 1) * L],
                                             ACTF.Relu, bias=b1a_t[m][:], scale=1.0)
                        ha[s].append(h)
                hT = [[], []]
                for m, (m0, m1) in enumerate(HM):
                    pb = ps_pair(psMM, "pb")
                    for s in range(2):
                        for k2 in range(2):
                            nc.tensor.matmul(pb[:m1 - m0, s * L:(s + 1) * L],
                                             W2a_t[k2][:, m0:m1], ha[s][k2][:],
                                             start=(k2 == 0), stop=(k2 == 1))
                    for s in range(2):
                        h = hp.tile([m1 - m0, L], BF, name=f"hT{s}{m}", tag=f"hT{s}{m}")
                        nc.scalar.activation(h[:], pb[:m1 - m0, s * L:(s + 1) * L],
                                             ACTF.Relu, bias=b2a_t[m][:], scale=1.0)
                        hT[s].append(h)

                # scores direction A only: e[i, j]; u0 = exp(e + logmask1[i])
                ep = ps_pair(psS, "score")
                for ic in range(2):
                    for m in range(2):
                        nc.tensor.matmul(ep[:, ic * L:(ic + 1) * L],
                                         hT[0][m][:, ic * P:(ic + 1) * P], hT[1][m][:],
                                         start=(m == 0), stop=(m == 1))
                u0 = sm.tile([P, 2 * L], BF, name="u0", tag="u0")
                for ic in range(2):
                    nc.scalar.activation(u0[:, ic * L:(ic + 1) * L], ep[:, ic * L:(ic + 1) * L],
                                         ACTF.Exp, bias=lmT[0][ic][:, b:b + 1], scale=1.0)
                # u1 = transpose(u0)  [j, i] (mask1 rides along on the free dim)
                tpu = ps_tr("u1tp")
                for jc in range(2):
                    for ic in range(2):
                        nc.tensor.transpose(tpu[:, jc * L + ic * P:jc * L + (ic + 1) * P],
                                            u0[:, ic * L + jc * P:ic * L + (jc + 1) * P], ident_b[:])
                u1 = sm.tile([P, 2 * L], BF, name="u1", tag="u1")
                nc.any.tensor_copy(u1[:], tpu[:, 0:2 * L])

                # attention sums + denominators.  dir A (alphas -> sentence-2
                # compare): lhsT = e1R cols, rhs = u0.  dir B (betas): lhsT =
                # e2m, rhs = u1.  Per direction: pair0 = [den | q2], pair1 =
                # [q0 | q1].
                de_dbg, rm_dbg = [], []
                for d in range(2):
                    lhs = eR[0] if d == 0 else e2m
                    rhs = u0 if d == 0 else u1
                    s = 1 - d
                    pair0 = ps_pair(psA, f"q_a{d}")
                    # den: M=1, ones col (dir A) / mask2 col (dir B)
                    for c in range(2):
                        lc = ones_col_b[:] if d == 0 else mTb[1][c][:, b:b + 1]
                        nc.tensor.matmul(pair0[0:1, 0:L], lc, rhs[:, c * L:(c + 1) * L],
                                         start=(c == 0), stop=(c == 1))
                    # q2 = attn cols 192:300
                    for c in range(2):
                        nc.tensor.matmul(pair0[0:108, L:2 * L], lhs[c][:, 192:300],
                                         rhs[:, c * L:(c + 1) * L], start=(c == 0), stop=(c == 1))
                    # normalizer from den
                    de = sm.tile([1, L], F32, name=f"de{d}", tag=f"de{d}")
                    nc.vector.tensor_scalar(de[:], pair0[0:1, 0:L], 1e-20, None, op0=ALU.add)
                    rc = sm.tile([1, L], F32, name=f"rc{d}", tag=f"rc{d}")
                    nc.vector.reciprocal(rc[:], de[:])
                    rm = sm.tile([1, L], BF, name=f"rm{d}", tag=f"rm{d}")
                    # dir A output cols masked by mask2[j]; dir B by mask1[i]
                    nc.vector.tensor_tensor(rm[:], rc[:], mrow[1 - d][:], op=ALU.mult)
                    rp = ps_pair(psS, "score")
                    nc.tensor.matmul(rp[:, 0:L], ones_row_b[:], rm[:], start=True, stop=True)
                    rb = sm.tile([P, L], BF, name=f"Rb{d}", tag=f"Rb{d}")
                    nc.any.tensor_copy(rb[:], rp[:, 0:L])
                    # q0 (cols 0:64 at partition offset 64) | q1 (cols 64:192)
                    pair1 = ps_pair(psA, f"q_b{d}")
                    for c in range(2):
                        nc.tensor.matmul(pair1[64:128, 0:L], lhs[c][:, 0:64],
                                         rhs[:, c * L:(c + 1) * L], start=(c == 0), stop=(c == 1))
                    for c in range(2):
                        nc.tensor.matmul(pair1[0:128, L:2 * L], lhs[c][:, 64:192],
                                         rhs[:, c * L:(c + 1) * L], start=(c == 0), stop=(c == 1))
                    # normalize into x tiles
                    nc.vector.tensor_tensor(xt[s][64:128, 2 * L:3 * L], pair1[64:128, 0:L],
                                            rb[64:128, :], op=ALU.mult)
                    nc.vector.tensor_tensor(xt[s][:, 3 * L:4 * L], pair1[:, L:2 * L],
                                            rb[:], op=ALU.mult)
                    nc.vector.tensor_tensor(xt[s][0:108, 4 * L:5 * L], pair0[0:108, L:2 * L],
                                            rb[0:108, :], op=ALU.mult)
                    de_dbg.append(de)
                    rm_dbg.append(rm)

                if dbg and b == 0:
                    nc.sync.dma_start(dbg_d['eR00'][:], eR[0][0][:])
                    nc.sync.dma_start(dbg_d['xt0'][:], xt[0][:])
                    nc.sync.dma_start(dbg_d['xt1'][:], xt[1][:])
                    nc.sync.dma_start(dbg_d['u0'][:], u0[:])
                    nc.sync.dma_start(dbg_d['u1'][:], u1[:])
                    nc.sync.dma_start(dbg_d['hT00'][:], hT[0][0][:])
                    nc.sync.dma_start(dbg_d['de0'][:], de_dbg[0][:])
                    nc.sync.dma_start(dbg_d['de1'][:], de_dbg[1][:])
                    nc.sync.dma_start(dbg_d['rm0'][:], rm_dbg[0][:])
                    nc.sync.dma_start(dbg_d['rm1'][:], rm_dbg[1][:])

                # compare MLP
                r1 = [[], []]
                for m, (m0, m1) in enumerate(HM):
                    pc = ps_pair(psMM, "pa")
                    for s in range(2):
                        for k in range(5):
                            nc.tensor.matmul(pc[:m1 - m0, s * L:(s + 1) * L],
                                             W1c_t[k][:, m0:m1],
                                             xt[s][0:XCHUNK[k], k * L:(k + 1) * L],
                                             start=(k == 0), stop=(k == 4))
                    for s in range(2):
                        r = hp.tile([m1 - m0, L], BF, name=f"r1{s}{m}", tag=f"r1{s}{m}")
                        nc.scalar.activation(r[:], pc[:m1 - m0, s * L:(s + 1) * L],
                                             ACTF.Relu, bias=b1c_t[m][:], scale=1.0)
                        r1[s].append(r)
                for m, (m0, m1) in enumerate(HM):
                    pd = ps_pair(psMM, "pb")
                    for s in range(2):
                        for k2 in range(2):
                            nc.tensor.matmul(pd[:m1 - m0, s * L:(s + 1) * L],
                                             W2c_t[k2][:, m0:m1], r1[s][k2][:],
                                             start=(k2 == 0), stop=False)
                        # += logmask[s][j] on every partition: masked cols relu to 0
                        nc.tensor.matmul(pd[:m1 - m0, s * L:(s + 1) * L],
                                         ones_row_b[:, 0:m1 - m0], lmrow[s][:],
                                         start=False, stop=True)
                    for s in range(2):
                        scr = hp.tile([m1 - m0, L], BF, name=f"scr{s}{m}", tag=f"scr{s}{m}")
                        nc.scalar.activation(scr[:], pd[:m1 - m0, s * L:(s + 1) * L],
                                             ACTF.Relu, bias=b2c_t[m][:],
                                             scale=1.0, accum_out=v_all[s][m][:, b:b + 1])

            # ---------------- aggregate ----------------
            if dbg:
                nc.sync.dma_start(dbg_d['v00'][:], v_all[0][0][:])
            vb = []
            for s in range(2):
                for m, (m0, m1) in enumerate(HM):
                    t = const.tile([m1 - m0, nb], BF, name=f"vb{s}{m}", tag=f"vb{s}{m}")
                    nc.vector.tensor_copy(t[:], v_all[s][m][:])
                    vb.append(t)
            g1 = []
            gp = ps_pair(psMM, "pa")
            for m, (m0, m1) in enumerate(HM):
                for k in range(4):
                    nc.tensor.matmul(gp[:m1 - m0, m * nb:(m + 1) * nb],
                                     W1g_t[k][:, m0:m1], vb[k][:],
                                     start=(k == 0), stop=(k == 3))
            for m, (m0, m1) in enumerate(HM):
                g = const.tile([m1 - m0, nb], BF, name=f"g1{m}", tag=f"g1{m}")
                nc.scalar.activation(g[:], gp[:m1 - m0, m * nb:(m + 1) * nb],
                                     ACTF.Relu, bias=b1g_t[m][:], scale=1.0)
                g1.append(g)
            op = ps_pair(psMM, "pb")
            for k2 in range(2):
                nc.tensor.matmul(op[0:2, 0:nb], W2g_t[k2][:], g1[k2][:],
                                 start=(k2 == 0), stop=(k2 == 1))
            osb = const.tile([2, nb], F32, name="osb", tag="osb")
            nc.scalar.activation(osb[:], op[0:2, 0:nb], ACTF.Identity, bias=b2g_t[:], scale=1.0)
            nc.sync.dma_start(out_d[:].rearrange("b o -> o b"), osb[:])

    nc.compile()
    return nc


def _shard_inputs(inputs, nb=BC, ncores=NCORES):
    import ml_dtypes
    bf16 = ml_dtypes.bfloat16
    f = np.ascontiguousarray

    emb_bf = f(inputs['emb'].astype(bf16))
    # W1c packed into 5 chunks of 128 rows: [0:256] e-rows, chunk2 = 44 e-tail
    # rows + 20 zero rows + 64 beta rows, then beta rows 64:192, 192:300.
    W1c = inputs['W1c'].astype(np.float32)
    W1c_p = np.zeros((640, HIDDEN), np.float32)
    W1c_p[0:256] = W1c[0:256]
    W1c_p[256:300] = W1c[256:300]
    W1c_p[320:384] = W1c[300:364]
    W1c_p[384:512] = W1c[364:492]
    W1c_p[512:620] = W1c[492:600]
    wb = {k: f(inputs[k].astype(bf16)) for k in ('W1a', 'W2a', 'W2c', 'W1g', 'W2g')}
    wb['W1c'] = f(W1c_p.astype(bf16))

    maps = []
    for c in range(ncores):
        sl = slice(c * nb, (c + 1) * nb)
        maps.append(dict(
            emb=emb_bf,
            s1=f(inputs['s1'][sl].astype(np.int32)),
            s2=f(inputs['s2'][sl].astype(np.int32)),
            len1=f(inputs['len1'][sl].reshape(nb, 1).astype(np.int32)),
            len2=f(inputs['len2'][sl].reshape(nb, 1).astype(np.int32)),
            b1a=f(inputs['b1a'].reshape(-1, 1).astype(np.float32)),
            b2a=f(inputs['b2a'].reshape(-1, 1).astype(np.float32)),
            b1c=f(inputs['b1c'].reshape(-1, 1).astype(np.float32)),
            b2c=f(inputs['b2c'].reshape(-1, 1).astype(np.float32)),
            b1g=f(inputs['b1g'].reshape(-1, 1).astype(np.float32)),
            b2g=f(inputs['b2g'].reshape(-1, 1).astype(np.float32)),
            **wb,
        ))
    return maps


def kernel(**inputs):
    from concourse.bass_utils import run_bass_kernel_spmd
    if 'prog' not in _prog_cache:
        _prog_cache['prog'] = build_program(BC)
    nc = _prog_cache['prog']
    in_maps = _shard_inputs(inputs)
    res = run_bass_kernel_spmd(nc, in_maps, core_ids=list(range(NCORES)))
    out = np.concatenate([res.results[c]["out"] for c in range(NCORES)], axis=0)
    return out.astype(np.float32)


# revision 20
# speedup vs baseline: 2.2400x; 1.4459x over previous
"""Trainium2 Bass kernel for DecomposableAttention (B=512, L=256, V=50000, E=300, H=200).

Data-parallel over batch across 8 cores (64 batches/core).  All matmuls bf16
(1 cycle/row), fp32 PSUM.  Per batch:

- indirect-DMA gather of bf16 embedding rows (table converted host-side) with a
  ones/mask column appended for the softmax denominators
- PE transposes to the E-on-partitions layout; both sentences packed side by
  side in one [128, 2560] x tile so every MLP matmul/activation runs once with
  a 512-wide free dim
- attend MLP -> scores (one direction) -> exp without the max stabilizer
  (scores bounded ~11; the stabilizer cancels exactly after normalization);
  the reverse-direction exp'd scores come from a PE transpose of u0
- attention sums in [token, 301] layout: the denominator lands in column 300 of
  the same matmul, so 1/(den+eps) is a [128,1] per-partition op and the
  normalization + output masking fuse into one tensor_scalar; the result is
  PE-transposed into the compare layout
- compare MLP over 5 packed 128-row K-chunks; masked column-sum fused via a
  K=1 logmask matmul into PSUM + the activation engine's accum_out

PSUM (8 banks x 2KB): psT 2 (bf16 transpose staging), psMM 2 (MLP pairs),
psS 2 (scores), psA 2 (attention sums).  Accumulation groups never interleave
within a bank (hardware breaks interleaved groups) and tiles never straddle
bank boundaries.
"""
import sys

if '/opt/trn_rl_repo' not in sys.path:
    sys.path.insert(0, '/opt/trn_rl_repo')

import numpy as np

B, L, VOCAB, EMBED, HIDDEN = 512, 256, 50000, 300, 200
NCORES = 8
BC = B // NCORES  # batches per core
P = 128
E1 = EMBED + 1    # emb cols + ones/mask column

_prog_cache = {}

# K-chunk layout of the 600-dim compare input [e (300); beta (300)].
# Chunk 2 mixes the e tail (44 rows) + 20 zero rows + beta[0:64] at partition
# offset 64; W1c is host-packed to match.
XCHUNK = [128, 128, 128, 128, 108]
BCHUNK = [(0, 64, 64), (64, 192, 0), (192, 300, 0)]  # beta col ranges + partition offset


def build_program(nb=BC, ndev=NCORES, dbg=False):
    import concourse.bass as bass
    import concourse.bacc as bacc
    import concourse.tile as tile
    import concourse.mybir as mybir
    from concourse.masks import make_identity

    F32 = mybir.dt.float32
    BF = mybir.dt.bfloat16
    I32 = mybir.dt.int32
    ALU = mybir.AluOpType
    ACTF = mybir.ActivationFunctionType
    EK = [(0, 128), (128, 256), (256, 300)]   # E contraction chunks
    HM = [(0, 128), (128, 200)]               # H chunks (128, 72)

    nc = bacc.Bacc("TRN2", num_devices=ndev)

    emb_d = nc.dram_tensor("emb", [VOCAB, EMBED], BF, kind="ExternalInput")
    s1_d = nc.dram_tensor("s1", [nb, L], I32, kind="ExternalInput")
    s2_d = nc.dram_tensor("s2", [nb, L], I32, kind="ExternalInput")
    len1_d = nc.dram_tensor("len1", [nb, 1], I32, kind="ExternalInput")
    len2_d = nc.dram_tensor("len2", [nb, 1], I32, kind="ExternalInput")
    W1a_d = nc.dram_tensor("W1a", [EMBED, HIDDEN], BF, kind="ExternalInput")
    W2a_d = nc.dram_tensor("W2a", [HIDDEN, HIDDEN], BF, kind="ExternalInput")
    W1c_d = nc.dram_tensor("W1c", [640, HIDDEN], BF, kind="ExternalInput")   # host-packed 5x128
    W2c_d = nc.dram_tensor("W2c", [HIDDEN, HIDDEN], BF, kind="ExternalInput")
    W1g_d = nc.dram_tensor("W1g", [2 * HIDDEN, HIDDEN], BF, kind="ExternalInput")
    W2g_d = nc.dram_tensor("W2g", [HIDDEN, 2], BF, kind="ExternalInput")
    b1a_d = nc.dram_tensor("b1a", [HIDDEN, 1], F32, kind="ExternalInput")
    b2a_d = nc.dram_tensor("b2a", [HIDDEN, 1], F32, kind="ExternalInput")
    b1c_d = nc.dram_tensor("b1c", [HIDDEN, 1], F32, kind="ExternalInput")
    b2c_d = nc.dram_tensor("b2c", [HIDDEN, 1], F32, kind="ExternalInput")
    b1g_d = nc.dram_tensor("b1g", [HIDDEN, 1], F32, kind="ExternalInput")
    b2g_d = nc.dram_tensor("b2g", [2, 1], F32, kind="ExternalInput")
    out_d = nc.dram_tensor("out", [nb, 2], F32, kind="ExternalOutput")
    if dbg:
        dbg_d = {
            'XT': nc.dram_tensor("dbg_XT", [P, 10 * L], BF, kind="ExternalOutput"),
            'u0': nc.dram_tensor("dbg_u0", [P, 2 * L], BF, kind="ExternalOutput"),
            'u1': nc.dram_tensor("dbg_u1", [P, 2 * L], BF, kind="ExternalOutput"),
            'hT0': nc.dram_tensor("dbg_hT0", [P, 2 * L], BF, kind="ExternalOutput"),
            'v00': nc.dram_tensor("dbg_v00", [P, nb], F32, kind="ExternalOutput"),
        }

    with tile.TileContext(nc) as tc:
        import contextlib
        ctx = contextlib.ExitStack()
        with ctx:
            const = ctx.enter_context(tc.tile_pool(name="const", bufs=1))
            psT = ctx.enter_context(tc.tile_pool(name="psT", bufs=2, space="PSUM"))
            psMM = ctx.enter_context(tc.tile_pool(name="psMM", bufs=2, space="PSUM"))
            psS = ctx.enter_context(tc.tile_pool(name="psS", bufs=2, space="PSUM"))
            psA = ctx.enter_context(tc.tile_pool(name="psA", bufs=2, space="PSUM"))
            gat = ctx.enter_context(tc.tile_pool(name="gat", bufs=3))
            xtp = ctx.enter_context(tc.tile_pool(name="xtp", bufs=2))
            hp = ctx.enter_context(tc.tile_pool(name="hp", bufs=3))
            sm = ctx.enter_context(tc.tile_pool(name="sm", bufs=3))

            def ps_pair(pool, nm):
                return pool.tile([P, 512], F32, name=nm, tag=pool.name)

            def ps_tr(nm):
                # full 2KB so every pool buffer is PSUM-bank aligned
                return psT.tile([P, 1024], BF, name=nm, tag="psT")

            # ---------------- constants ----------------
            ident = const.tile([P, P], F32)
            make_identity(nc, ident[:])
            ident_b = const.tile([P, P], BF)
            nc.vector.tensor_copy(ident_b[:], ident[:])
            ones_row_b = const.tile([1, P], BF)
            nc.vector.memset(ones_row_b[:], 1.0)

            # weights (bf16, direct DMA)
            W1a_t = [const.tile([k1 - k0, HIDDEN], BF, name=f"W1a{i}", tag=f"W1a{i}")
                     for i, (k0, k1) in enumerate(EK)]
            for i, (k0, k1) in enumerate(EK):
                nc.sync.dma_start(W1a_t[i][:], W1a_d[k0:k1, :])
            W2a_t = [const.tile([m1 - m0, HIDDEN], BF, name=f"W2a{i}", tag=f"W2a{i}")
                     for i, (m0, m1) in enumerate(HM)]
            for i, (m0, m1) in enumerate(HM):
                nc.sync.dma_start(W2a_t[i][:], W2a_d[m0:m1, :])
            W1c_t = [const.tile([k, HIDDEN], BF, name=f"W1c{i}", tag=f"W1c{i}")
                     for i, k in enumerate(XCHUNK)]
            for i in range(5):
                nc.sync.dma_start(W1c_t[i][:], W1c_d[i * 128:i * 128 + XCHUNK[i], :])
            W2c_t = [const.tile([m1 - m0, HIDDEN], BF, name=f"W2c{i}", tag=f"W2c{i}")
                     for i, (m0, m1) in enumerate(HM)]
            for i, (m0, m1) in enumerate(HM):
                nc.sync.dma_start(W2c_t[i][:], W2c_d[m0:m1, :])
            GK = [(0, 128), (128, 200), (200, 328), (328, 400)]
            W1g_t = [const.tile([k1 - k0, HIDDEN], BF, name=f"W1g{i}", tag=f"W1g{i}")
                     for i, (k0, k1) in enumerate(GK)]
            for i, (k0, k1) in enumerate(GK):
                nc.sync.dma_start(W1g_t[i][:], W1g_d[k0:k1, :])
            W2g_t = [const.tile([m1 - m0, 2], BF, name=f"W2g{i}", tag=f"W2g{i}")
                     for i, (m0, m1) in enumerate(HM)]
            for i, (m0, m1) in enumerate(HM):
                nc.sync.dma_start(W2g_t[i][:], W2g_d[m0:m1, :])

            def bias2(d, nm):
                t = [const.tile([m1 - m0, 1], F32, name=f"b{nm}{i}", tag=f"b{nm}{i}")
                     for i, (m0, m1) in enumerate(HM)]
                for i, (m0, m1) in enumerate(HM):
                    nc.sync.dma_start(t[i][:], d[m0:m1, :])
                return t
            b1a_t, b2a_t = bias2(b1a_d, "1a"), bias2(b2a_d, "2a")
            b1c_t, b2c_t = bias2(b1c_d, "1c"), bias2(b2c_d, "2c")
            b1g_t = bias2(b1g_d, "1g")
            b2g_t = const.tile([2, 1], F32)
            nc.sync.dma_start(b2g_t[:], b2g_d[:])

            # masks / lengths
            len_t = []
            len_f = []
            for s, ld in ((0, len1_d), (1, len2_d)):
                lt = const.tile([nb, 1], I32, name=f"len{s}", tag=f"len{s}")
                nc.sync.dma_start(lt[:], ld[:])
                len_t.append(lt)
                lf = const.tile([1, nb], I32, name=f"lenf{s}", tag=f"lenf{s}")
                nc.sync.dma_start(lf[:], ld[:].rearrange("n one -> one n"))
                len_f.append(lf)
            iota_t = const.tile([nb, L], I32)
            nc.gpsimd.iota(iota_t[:], pattern=[[1, L]], base=0, channel_multiplier=0)
            iota2 = const.tile([1, 2 * L], I32)   # 0..255 twice
            nc.gpsimd.iota(iota2[:], pattern=[[0, 2], [1, L]], base=0, channel_multiplier=0)

            lmT = []   # transposed logmask1 [128, nb] f32, 2 chunks (exp bias)
            mTb = []   # transposed 0/1 masks [128, nb] bf16, 2 chunks per sentence
            mTf = []   # same masks in f32 (tensor_scalar scalar2 operand)
            for s in range(2):
                m = const.tile([nb, L], F32, name=f"mask{s}", tag=f"mask{s}")
                nc.vector.tensor_tensor(m[:], iota_t[:], len_t[s][:].to_broadcast([nb, L]), op=ALU.is_lt)
                lts, mts, mfs = [], [], []
                for c in range(2):
                    if s == 0:
                        lm = const.tile([nb, L], F32, name="lm0", tag="lm0")
                        nc.vector.tensor_scalar(lm[:], m[:], 1.0, 30000.0,
                                                op0=ALU.subtract, op1=ALU.mult)
                        tp = ps_pair(psS, "setup_tp")
                        nc.tensor.transpose(tp[:, 0:nb], lm[:, c * P:(c + 1) * P], ident[:nb, :nb])
                        lt = const.tile([P, nb], F32, name=f"lmT{s}{c}", tag=f"lmT{s}{c}")
                        nc.vector.tensor_copy(lt[:], tp[:, 0:nb])
                        lts.append(lt)
                    tp2 = ps_pair(psS, "setup_tp")
                    nc.tensor.transpose(tp2[:, 0:nb], m[:, c * P:(c + 1) * P], ident[:nb, :nb])
                    mt = const.tile([P, nb], BF, name=f"mTb{s}{c}", tag=f"mTb{s}{c}")
                    nc.vector.tensor_copy(mt[:], tp2[:, 0:nb])
                    mts.append(mt)
                    mf = const.tile([P, nb], F32, name=f"mTf{s}{c}", tag=f"mTf{s}{c}")
                    nc.vector.tensor_copy(mf[:], tp2[:, 0:nb])
                    mfs.append(mf)
                lmT.append(lts)
                mTb.append(mts)
                mTf.append(mfs)

            # token indices, transposed to [128, nb] int32 per chunk
            sT = []
            for s, sd in ((0, s1_d), (1, s2_d)):
                st = const.tile([nb, L], I32, name=f"s{s}", tag=f"s{s}")
                nc.sync.dma_start(st[:], sd[:])
                sf = const.tile([nb, L], F32, name=f"sf{s}", tag=f"sf{s}")
                nc.vector.tensor_copy(sf[:], st[:])
                chunks = []
                for c in range(2):
                    tp = ps_pair(psS, "setup_tp")
                    nc.tensor.transpose(tp[:, 0:nb], sf[:, c * P:(c + 1) * P], ident[:nb, :nb])
                    tf = const.tile([P, nb], F32, name=f"sTf{s}{c}", tag=f"sTf{s}{c}")
                    nc.vector.tensor_copy(tf[:], tp[:, 0:nb])
                    ti = const.tile([P, nb], I32, name=f"sTi{s}{c}", tag=f"sTi{s}{c}")
                    nc.vector.tensor_copy(ti[:], tf[:])
                    chunks.append(ti)
                sT.append(chunks)

            # v accumulators [(128|72), nb] per H-chunk per sentence
            v_all = [[const.tile([m1 - m0, nb], F32, name=f"v{s}{m}", tag=f"v{s}{m}")
                      for m, (m0, m1) in enumerate(HM)] for s in range(2)]

            # ---------------- per-batch loop ----------------
            for b in range(nb):
                # merged logmask row [1, 512] (both sentences) for compare-L2
                lens2 = sm.tile([1, 2 * L], I32, name="lens2", tag="lens2")
                for s in range(2):
                    nc.vector.tensor_copy(lens2[:, s * L:(s + 1) * L],
                                          len_f[s][:, b:b + 1].to_broadcast([1, L]))
                mrow2 = sm.tile([1, 2 * L], F32, name="mrow2", tag="mrow2")
                nc.vector.tensor_tensor(mrow2[:], iota2[:], lens2[:], op=ALU.is_lt)
                lmrow2 = sm.tile([1, 2 * L], BF, name="lmrow2", tag="lmrow2")
                nc.vector.tensor_scalar(lmrow2[:], mrow2[:], 1.0, 30000.0,
                                        op0=ALU.subtract, op1=ALU.mult)

                # gather embeddings (bf16 rows); col 300 <- 1.0
                eR = [[], []]
                for s in range(2):
                    for c in range(2):
                        er = gat.tile([P, E1], BF, name=f"eR{s}{c}", tag=f"eR{s}{c}")
                        nc.gpsimd.indirect_dma_start(
                            out=er[:, 0:EMBED], out_offset=None, in_=emb_d[:],
                            in_offset=bass.IndirectOffsetOnAxis(ap=sT[s][c][:, b:b + 1], axis=0),
                        )
                        nc.vector.memset(er[:, EMBED:E1], 1.0)
                        eR[s].append(er)
                # e2m = e2R * mask2 col (col 300 becomes the mask2 column)
                e2m = []
                for c in range(2):
                    em = gat.tile([P, E1], BF, name=f"e2m{c}", tag=f"e2m{c}")
                    nc.vector.tensor_tensor(em[:], eR[1][c][:],
                                            mTb[1][c][:, b:b + 1].to_broadcast([P, E1]), op=ALU.mult)
                    e2m.append(em)

                # x input [128, 10*256] bf16: chunk k at [k*512, (k+1)*512),
                # sentence s at +s*256.  Chunks 0,1 = eT rows 0:256; chunk2 =
                # eT 256:300 | 20 zero rows | beta 0:64 at partition 64;
                # chunk3 = beta 64:192; chunk4 = beta 192:300.
                XT = xtp.tile([P, 10 * L], BF, name="XT", tag="XT")
                for s in range(2):
                    nc.vector.memset(XT[0:64, 4 * L + s * L:4 * L + (s + 1) * L], 0.0)
                    tp = ps_tr(f"eTtp{s}")
                    for k, (k0, k1) in enumerate(EK):
                        for c in range(2):
                            nc.tensor.transpose(tp[:k1 - k0, k * L + c * P:k * L + (c + 1) * P],
                                                eR[s][c][:, k0:k1], ident_b[:])
                    for k in range(2):
                        nc.any.tensor_copy(XT[:, 2 * k * L + s * L:2 * k * L + (s + 1) * L],
                                           tp[:, k * L:(k + 1) * L])
                    nc.any.tensor_copy(XT[0:44, 4 * L + s * L:4 * L + (s + 1) * L],
                                       tp[0:44, 2 * L:3 * L])

                # attend MLP (both sentences per matmul, N=512)
                ha = []
                for m, (m0, m1) in enumerate(HM):
                    pa = ps_pair(psMM, "pa")
                    for k, (k0, k1) in enumerate(EK):
                        nc.tensor.matmul(pa[:m1 - m0, :], W1a_t[k][:, m0:m1],
                                         XT[0:k1 - k0, 2 * k * L:2 * (k + 1) * L],
                                         start=(k == 0), stop=(k == 2))
                    h = hp.tile([m1 - m0, 2 * L], BF, name=f"ha{m}", tag=f"ha{m}")
                    nc.scalar.activation(h[:], pa[:m1 - m0, :], ACTF.Relu,
                                         bias=b1a_t[m][:], scale=1.0)
                    ha.append(h)
                hT = []
                for m, (m0, m1) in enumerate(HM):
                    pb = ps_pair(psMM, "pb")
                    for k2 in range(2):
                        nc.tensor.matmul(pb[:m1 - m0, :], W2a_t[k2][:, m0:m1], ha[k2][:],
                                         start=(k2 == 0), stop=(k2 == 1))
                    h = hp.tile([m1 - m0, 2 * L], BF, name=f"hT{m}", tag=f"hT{m}")
                    nc.scalar.activation(h[:], pb[:m1 - m0, :], ACTF.Relu,
                                         bias=b2a_t[m][:], scale=1.0)
                    hT.append(h)

                # scores: e[i, j] = sum_m h1[m, i] h2[m, j]; u0 = exp(e + logmask1[i])
                ep = ps_pair(psS, "score")
                for ic in range(2):
                    for m in range(2):
                        nc.tensor.matmul(ep[:, ic * L:(ic + 1) * L],
                                         hT[m][:, ic * P:(ic + 1) * P], hT[m][:, L:2 * L],
                                         start=(m == 0), stop=(m == 1))
                u0 = sm.tile([P, 2 * L], BF, name="u0", tag="u0")
                for ic in range(2):
                    nc.scalar.activation(u0[:, ic * L:(ic + 1) * L], ep[:, ic * L:(ic + 1) * L],
                                         ACTF.Exp, bias=lmT[0][ic][:, b:b + 1], scale=1.0)
                # u1 = transpose(u0) [j, i]
                tpu = ps_tr("u1tp")
                for jc in range(2):
                    for ic in range(2):
                        nc.tensor.transpose(tpu[:, jc * L + ic * P:jc * L + (ic + 1) * P],
                                            u0[:, ic * L + jc * P:ic * L + (jc + 1) * P], ident_b[:])
                u1 = sm.tile([P, 2 * L], BF, name="u1", tag="u1")
                nc.any.tensor_copy(u1[:], tpu[:, 0:2 * L])

                # attention sums in [token, 301] layout; den = col 300.
                # dir A (alphas -> sentence-2 compare): out [j, e'] = sum_i
                # u0[i, j]^T e1R[i, :]; dir B (betas): out [i, e'] via u1/e2m.
                for d in range(2):
                    u_t = u0 if d == 0 else u1
                    rhs = eR[0] if d == 0 else e2m
                    s = 1 - d
                    tt = ps_tr(f"bt{d}")
                    for t_ in range(2):   # output token chunk (j for A, i for B)
                        q = ps_pair(psA, f"q{d}{t_}")
                        for c in range(2):  # contraction chunk (i for A, j for B)
                            nc.tensor.matmul(q[:, 0:E1],
                                             u_t[:, c * L + t_ * P:c * L + (t_ + 1) * P],
                                             rhs[c][:, 0:E1], start=(c == 0), stop=(c == 1))
                        # rcp = 1 / (den + eps)  ([128,1] per-partition ops)
                        dcol = sm.tile([P, 1], F32, name=f"dc{d}{t_}", tag=f"dc{d}{t_}")
                        nc.vector.tensor_scalar(dcol[:], q[:, EMBED:E1], 1e-20, None, op0=ALU.add)
                        rcp = sm.tile([P, 1], F32, name=f"rcp{d}{t_}", tag=f"rcp{d}{t_}")
                        nc.vector.reciprocal(rcp[:], dcol[:])
                        # normalized + output-masked rows, bf16
                        # dir A masks by mask2[j]; dir B by mask1[i]
                        nrm = sm.tile([P, EMBED], BF, name=f"nrm{d}{t_}", tag=f"nrm{d}{t_}")
                        nc.vector.tensor_scalar(nrm[:], q[:, 0:EMBED], rcp[:],
                                                mTf[1 - d][t_][:, b:b + 1],
                                                op0=ALU.mult, op1=ALU.mult)
                        # transpose into compare layout
                        for t, (c0, c1, poff) in enumerate(BCHUNK):
                            nc.tensor.transpose(
                                tt[poff:poff + (c1 - c0), t * L + t_ * P:t * L + (t_ + 1) * P],
                                nrm[:, c0:c1], ident_b[:])
                    # pack both token chunks into XT
                    nc.any.tensor_copy(XT[64:128, 4 * L + s * L:4 * L + (s + 1) * L],
                                       tt[64:128, 0:L])
                    nc.any.tensor_copy(XT[:, 6 * L + s * L:6 * L + (s + 1) * L],
                                       tt[:, L:2 * L])
                    nc.any.tensor_copy(XT[0:108, 8 * L + s * L:8 * L + (s + 1) * L],
                                       tt[0:108, 2 * L:3 * L])

                # compare MLP (N=512)
                r1 = []
                for m, (m0, m1) in enumerate(HM):
                    pc = ps_pair(psMM, "pc")
                    for k in range(5):
                        nc.tensor.matmul(pc[:m1 - m0, :], W1c_t[k][:, m0:m1],
                                         XT[0:XCHUNK[k], 2 * k * L:2 * (k + 1) * L],
                                         start=(k == 0), stop=(k == 4))
                    r = hp.tile([m1 - m0, 2 * L], BF, name=f"r1{m}", tag=f"r1{m}")
                    nc.scalar.activation(r[:], pc[:m1 - m0, :], ACTF.Relu,
                                         bias=b1c_t[m][:], scale=1.0)
                    r1.append(r)
                for m, (m0, m1) in enumerate(HM):
                    pd = ps_pair(psMM, "pd")
                    for k2 in range(2):
                        nc.tensor.matmul(pd[:m1 - m0, :], W2c_t[k2][:, m0:m1], r1[k2][:],
                                         start=(k2 == 0), stop=False)
                    # += logmask[j] on every partition: masked cols relu to 0
                    nc.tensor.matmul(pd[:m1 - m0, :], ones_row_b[:, 0:m1 - m0],
                                     lmrow2[:], start=False, stop=True)
                    for s in range(2):
                        scr = hp.tile([m1 - m0, L], BF, name=f"scr{s}{m}", tag=f"scr{s}{m}")
                        nc.scalar.activation(scr[:], pd[:m1 - m0, s * L:(s + 1) * L],
                                             ACTF.Relu, bias=b2c_t[m][:],
                                             scale=1.0, accum_out=v_all[s][m][:, b:b + 1])

                if dbg and b == 0:
                    nc.sync.dma_start(dbg_d['XT'][:], XT[:])
                    nc.sync.dma_start(dbg_d['u0'][:], u0[:])
                    nc.sync.dma_start(dbg_d['u1'][:], u1[:])
                    nc.sync.dma_start(dbg_d['hT0'][:], hT[0][:])

            # ---------------- aggregate ----------------
            if dbg:
                nc.sync.dma_start(dbg_d['v00'][:], v_all[0][0][:])
            vb = []
            for s in range(2):
                for m, (m0, m1) in enumerate(HM):
                    t = const.tile([m1 - m0, nb], BF, name=f"vb{s}{m}", tag=f"vb{s}{m}")
                    nc.vector.tensor_copy(t[:], v_all[s][m][:])
                    vb.append(t)
            g1 = []
            gp = ps_pair(psMM, "pa")
            for m, (m0, m1) in enumerate(HM):
                for k in range(4):
                    nc.tensor.matmul(gp[:m1 - m0, m * nb:(m + 1) * nb],
                                     W1g_t[k][:, m0:m1], vb[k][:],
                                     start=(k == 0), stop=(k == 3))
            for m, (m0, m1) in enumerate(HM):
                g = const.tile([m1 - m0, nb], BF, name=f"g1{m}", tag=f"g1{m}")
                nc.scalar.activation(g[:], gp[:m1 - m0, m * nb:(m + 1) * nb],
                                     ACTF.Relu, bias=b1g_t[m][:], scale=1.0)
                g1.append(g)
            op = ps_pair(psMM, "pb")
            for k2 in range(2):
                nc.tensor.matmul(op[0:2, 0:nb], W2g_t[k2][:], g1[k2][:],
                                 start=(k2 == 0), stop=(k2 == 1))
            osb = const.tile([2, nb], F32, name="osb", tag="osb")
            nc.scalar.activation(osb[:], op[0:2, 0:nb], ACTF.Identity, bias=b2g_t[:], scale=1.0)
            nc.sync.dma_start(out_d[:].rearrange("b o -> o b"), osb[:])

    nc.compile()
    return nc


def _shard_inputs(inputs, nb=BC, ncores=NCORES):
    import ml_dtypes
    bf16 = ml_dtypes.bfloat16
    f = np.ascontiguousarray

    emb_bf = f(inputs['emb'].astype(bf16))
    # W1c packed into 5 chunks of 128 rows: [0:256] e-rows, chunk2 = 44 e-tail
    # rows + 20 zero rows + 64 beta rows, then beta rows 64:192, 192:300.
    W1c = inputs['W1c'].astype(np.float32)
    W1c_p = np.zeros((640, HIDDEN), np.float32)
    W1c_p[0:256] = W1c[0:256]
    W1c_p[256:300] = W1c[256:300]
    W1c_p[320:384] = W1c[300:364]
    W1c_p[384:512] = W1c[364:492]
    W1c_p[512:620] = W1c[492:600]
    wb = {k: f(inputs[k].astype(bf16)) for k in ('W1a', 'W2a', 'W2c', 'W1g', 'W2g')}
    wb['W1c'] = f(W1c_p.astype(bf16))

    maps = []
    for c in range(ncores):
        sl = slice(c * nb, (c + 1) * nb)
        maps.append(dict(
            emb=emb_bf,
            s1=f(inputs['s1'][sl].astype(np.int32)),
            s2=f(inputs['s2'][sl].astype(np.int32)),
            len1=f(inputs['len1'][sl].reshape(nb, 1).astype(np.int32)),
            len2=f(inputs['len2'][sl].reshape(nb, 1).astype(np.int32)),
            b1a=f(inputs['b1a'].reshape(-1, 1).astype(np.float32)),
            b2a=f(inputs['b2a'].reshape(-1, 1).astype(np.float32)),
            b1c=f(inputs['b1c'].reshape(-1, 1).astype(np.float32)),
            b2c=f(inputs['b2c'].reshape(-1, 1).astype(np.float32)),
            b1g=f(inputs['b1g'].reshape(-1, 1).astype(np.float32)),
            b2g=f(inputs['b2g'].reshape(-1, 1).astype(np.float32)),
            **wb,
        ))
    return maps


def kernel(**inputs):
    from concourse.bass_utils import run_bass_kernel_spmd
    if 'prog' not in _prog_cache:
        _prog_cache['prog'] = build_program(BC)
    nc = _prog_cache['prog']
    in_maps = _shard_inputs(inputs)
    res = run_bass_kernel_spmd(nc, in_maps, core_ids=list(range(NCORES)))
    out = np.concatenate([res.results[c]["out"] for c in range(NCORES)], axis=0)
    return out.astype(np.float32)


# revision 21
# speedup vs baseline: 4.0144x; 1.7922x over previous
"""Trainium2 Bass kernel for DecomposableAttention (B=512, L=256, V=50000, E=300, H=200).

Data-parallel over batch across 8 cores (64 batches/core).  All matmuls bf16
(1 cycle/row), fp32 PSUM.  Per batch:

- indirect-DMA gather of bf16 embedding rows (table converted host-side) with a
  ones/mask column appended for the softmax denominators
- PE transposes to the E-on-partitions layout; both sentences packed side by
  side in one [128, 2560] x tile so every MLP matmul/activation runs once with
  a 512-wide free dim
- attend MLP -> scores (one direction) -> exp without the max stabilizer
  (scores bounded ~11; the stabilizer cancels exactly after normalization);
  the reverse-direction exp'd scores come from a PE transpose of u0
- attention sums in [token, 301] layout: the denominator lands in column 300 of
  the same matmul, so 1/(den+eps) is a [128,1] per-partition op and the
  normalization + output masking fuse into one tensor_scalar; the result is
  PE-transposed into the compare layout
- compare MLP over 5 packed 128-row K-chunks; masked column-sum fused via a
  K=1 logmask matmul into PSUM + the activation engine's accum_out

PSUM (8 banks x 2KB): psT 2 (bf16 transpose staging), psMM 2 (MLP pairs),
psS 2 (scores), psA 2 (attention sums).  Accumulation groups never interleave
within a bank (hardware breaks interleaved groups) and tiles never straddle
bank boundaries.
"""
import sys

if '/opt/trn_rl_repo' not in sys.path:
    sys.path.insert(0, '/opt/trn_rl_repo')

import numpy as np

B, L, VOCAB, EMBED, HIDDEN = 512, 256, 50000, 300, 200
NCORES = 8
BC = B // NCORES  # batches per core
P = 128
E1 = EMBED + 1    # emb cols + ones/mask column

_prog_cache = {}

# K-chunk layout of the 600-dim compare input [e (300); beta (300)].
# Chunk 2 mixes the e tail (44 rows) + 20 zero rows + beta[0:64] at partition
# offset 64; W1c is host-packed to match.
XCHUNK = [128, 128, 128, 128, 108]
BCHUNK = [(0, 64, 64), (64, 192, 0), (192, 300, 0)]  # beta col ranges + partition offset


def build_program(nb=BC, ndev=NCORES, dbg=False):
    import concourse.bass as bass
    import concourse.bacc as bacc
    import concourse.tile as tile
    import concourse.mybir as mybir
    from concourse.masks import make_identity

    F32 = mybir.dt.float32
    BF = mybir.dt.bfloat16
    I32 = mybir.dt.int32
    ALU = mybir.AluOpType
    ACTF = mybir.ActivationFunctionType
    EK = [(0, 128), (128, 256), (256, 300)]   # E contraction chunks
    HM = [(0, 128), (128, 200)]               # H chunks (128, 72)

    nc = bacc.Bacc("TRN2", num_devices=ndev)

    emb_d = nc.dram_tensor("emb", [VOCAB, EMBED], BF, kind="ExternalInput")
    s1_d = nc.dram_tensor("s1", [nb, L], I32, kind="ExternalInput")
    s2_d = nc.dram_tensor("s2", [nb, L], I32, kind="ExternalInput")
    len1_d = nc.dram_tensor("len1", [nb, 1], I32, kind="ExternalInput")
    len2_d = nc.dram_tensor("len2", [nb, 1], I32, kind="ExternalInput")
    W1a_d = nc.dram_tensor("W1a", [EMBED, HIDDEN], BF, kind="ExternalInput")
    W2a_d = nc.dram_tensor("W2a", [HIDDEN, HIDDEN], BF, kind="ExternalInput")
    W1c_d = nc.dram_tensor("W1c", [640, HIDDEN], BF, kind="ExternalInput")   # host-packed 5x128
    W2c_d = nc.dram_tensor("W2c", [HIDDEN, HIDDEN], BF, kind="ExternalInput")
    W1g_d = nc.dram_tensor("W1g", [2 * HIDDEN, HIDDEN], BF, kind="ExternalInput")
    W2g_d = nc.dram_tensor("W2g", [HIDDEN, 2], BF, kind="ExternalInput")
    b1a_d = nc.dram_tensor("b1a", [HIDDEN, 1], F32, kind="ExternalInput")
    b2a_d = nc.dram_tensor("b2a", [HIDDEN, 1], F32, kind="ExternalInput")
    b1c_d = nc.dram_tensor("b1c", [HIDDEN, 1], F32, kind="ExternalInput")
    b2c_d = nc.dram_tensor("b2c", [HIDDEN, 1], F32, kind="ExternalInput")
    b1g_d = nc.dram_tensor("b1g", [HIDDEN, 1], F32, kind="ExternalInput")
    b2g_d = nc.dram_tensor("b2g", [2, 1], F32, kind="ExternalInput")
    out_d = nc.dram_tensor("out", [nb, 2], F32, kind="ExternalOutput")
    if dbg:
        dbg_d = {
            'XT': nc.dram_tensor("dbg_XT", [P, 10 * L], BF, kind="ExternalOutput"),
            'u0': nc.dram_tensor("dbg_u0", [P, 2 * L], BF, kind="ExternalOutput"),
            'u1': nc.dram_tensor("dbg_u1", [P, 2 * L], BF, kind="ExternalOutput"),
            'hT0': nc.dram_tensor("dbg_hT0", [P, 2 * L], BF, kind="ExternalOutput"),
            'v00': nc.dram_tensor("dbg_v00", [P, nb], F32, kind="ExternalOutput"),
        }

    with tile.TileContext(nc) as tc:
        import contextlib
        ctx = contextlib.ExitStack()
        with ctx:
            const = ctx.enter_context(tc.tile_pool(name="const", bufs=1))
            psT = ctx.enter_context(tc.tile_pool(name="psT", bufs=2, space="PSUM"))
            psMM = ctx.enter_context(tc.tile_pool(name="psMM", bufs=3, space="PSUM"))
            psS = ctx.enter_context(tc.tile_pool(name="psS", bufs=1, space="PSUM"))
            psA = ctx.enter_context(tc.tile_pool(name="psA", bufs=2, space="PSUM"))
            gat = ctx.enter_context(tc.tile_pool(name="gat", bufs=3))
            xtp = ctx.enter_context(tc.tile_pool(name="xtp", bufs=2))
            hp = ctx.enter_context(tc.tile_pool(name="hp", bufs=3))
            sm = ctx.enter_context(tc.tile_pool(name="sm", bufs=3))

            def ps_pair(pool, nm):
                return pool.tile([P, 512], F32, name=nm, tag=pool.name)

            def ps_tr(nm):
                # full 2KB so every pool buffer is PSUM-bank aligned
                return psT.tile([P, 1024], BF, name=nm, tag="psT")

            # ---------------- constants ----------------
            ident = const.tile([P, P], F32)
            make_identity(nc, ident[:])
            ident_b = const.tile([P, P], BF)
            nc.vector.tensor_copy(ident_b[:], ident[:])
            ones_row_b = const.tile([1, P], BF)
            nc.vector.memset(ones_row_b[:], 1.0)

            # weights (bf16, direct DMA)
            W1a_t = [const.tile([k1 - k0, HIDDEN], BF, name=f"W1a{i}", tag=f"W1a{i}")
                     for i, (k0, k1) in enumerate(EK)]
            for i, (k0, k1) in enumerate(EK):
                nc.sync.dma_start(W1a_t[i][:], W1a_d[k0:k1, :])
            W2a_t = [const.tile([m1 - m0, HIDDEN], BF, name=f"W2a{i}", tag=f"W2a{i}")
                     for i, (m0, m1) in enumerate(HM)]
            for i, (m0, m1) in enumerate(HM):
                nc.sync.dma_start(W2a_t[i][:], W2a_d[m0:m1, :])
            W1c_t = [const.tile([k, HIDDEN], BF, name=f"W1c{i}", tag=f"W1c{i}")
                     for i, k in enumerate(XCHUNK)]
            for i in range(5):
                nc.sync.dma_start(W1c_t[i][:], W1c_d[i * 128:i * 128 + XCHUNK[i], :])
            W2c_t = [const.tile([m1 - m0, HIDDEN], BF, name=f"W2c{i}", tag=f"W2c{i}")
                     for i, (m0, m1) in enumerate(HM)]
            for i, (m0, m1) in enumerate(HM):
                nc.sync.dma_start(W2c_t[i][:], W2c_d[m0:m1, :])
            GK = [(0, 128), (128, 200), (200, 328), (328, 400)]
            W1g_t = [const.tile([k1 - k0, HIDDEN], BF, name=f"W1g{i}", tag=f"W1g{i}")
                     for i, (k0, k1) in enumerate(GK)]
            for i, (k0, k1) in enumerate(GK):
                nc.sync.dma_start(W1g_t[i][:], W1g_d[k0:k1, :])
            W2g_t = [const.tile([m1 - m0, 2], BF, name=f"W2g{i}", tag=f"W2g{i}")
                     for i, (m0, m1) in enumerate(HM)]
            for i, (m0, m1) in enumerate(HM):
                nc.sync.dma_start(W2g_t[i][:], W2g_d[m0:m1, :])

            def bias2(d, nm):
                t = [const.tile([m1 - m0, 1], F32, name=f"b{nm}{i}", tag=f"b{nm}{i}")
                     for i, (m0, m1) in enumerate(HM)]
                for i, (m0, m1) in enumerate(HM):
                    nc.sync.dma_start(t[i][:], d[m0:m1, :])
                return t
            b1a_t, b2a_t = bias2(b1a_d, "1a"), bias2(b2a_d, "2a")
            b1c_t, b2c_t = bias2(b1c_d, "1c"), bias2(b2c_d, "2c")
            b1g_t = bias2(b1g_d, "1g")
            b2g_t = const.tile([2, 1], F32)
            nc.sync.dma_start(b2g_t[:], b2g_d[:])

            # masks / lengths
            len_t = []
            len_f = []
            for s, ld in ((0, len1_d), (1, len2_d)):
                lt = const.tile([nb, 1], I32, name=f"len{s}", tag=f"len{s}")
                nc.sync.dma_start(lt[:], ld[:])
                len_t.append(lt)
                lf = const.tile([1, nb], I32, name=f"lenf{s}", tag=f"lenf{s}")
                nc.sync.dma_start(lf[:], ld[:].rearrange("n one -> one n"))
                len_f.append(lf)
            iota_t = const.tile([nb, L], I32)
            nc.gpsimd.iota(iota_t[:], pattern=[[1, L]], base=0, channel_multiplier=0)
            iota2 = const.tile([1, 2 * L], I32)   # 0..255 twice
            nc.gpsimd.iota(iota2[:], pattern=[[0, 2], [1, L]], base=0, channel_multiplier=0)

            lmT = []   # transposed logmask1 [128, nb] f32, 2 chunks (exp bias)
            mTb = []   # transposed 0/1 masks [128, nb] bf16, 2 chunks per sentence
            mTf = []   # same masks in f32 (tensor_scalar scalar2 operand)
            for s in range(2):
                m = const.tile([nb, L], F32, name=f"mask{s}", tag=f"mask{s}")
                nc.vector.tensor_tensor(m[:], iota_t[:], len_t[s][:].to_broadcast([nb, L]), op=ALU.is_lt)
                lts, mts, mfs = [], [], []
                for c in range(2):
                    if s == 0:
                        lm = const.tile([nb, L], F32, name="lm0", tag="lm0")
                        nc.vector.tensor_scalar(lm[:], m[:], 1.0, 30000.0,
                                                op0=ALU.subtract, op1=ALU.mult)
                        tp = ps_pair(psS, "setup_tp")
                        nc.tensor.transpose(tp[:, 0:nb], lm[:, c * P:(c + 1) * P], ident[:nb, :nb])
                        lt = const.tile([P, nb], F32, name=f"lmT{s}{c}", tag=f"lmT{s}{c}")
                        nc.vector.tensor_copy(lt[:], tp[:, 0:nb])
                        lts.append(lt)
                    tp2 = ps_pair(psS, "setup_tp")
                    nc.tensor.transpose(tp2[:, 0:nb], m[:, c * P:(c + 1) * P], ident[:nb, :nb])
                    mt = const.tile([P, nb], BF, name=f"mTb{s}{c}", tag=f"mTb{s}{c}")
                    nc.vector.tensor_copy(mt[:], tp2[:, 0:nb])
                    mts.append(mt)
                    mf = const.tile([P, nb], F32, name=f"mTf{s}{c}", tag=f"mTf{s}{c}")
                    nc.vector.tensor_copy(mf[:], tp2[:, 0:nb])
                    mfs.append(mf)
                lmT.append(lts)
                mTb.append(mts)
                mTf.append(mfs)

            # token indices, transposed to [128, nb] int32 per chunk
            sT = []
            for s, sd in ((0, s1_d), (1, s2_d)):
                st = const.tile([nb, L], I32, name=f"s{s}", tag=f"s{s}")
                nc.sync.dma_start(st[:], sd[:])
                sf = const.tile([nb, L], F32, name=f"sf{s}", tag=f"sf{s}")
                nc.vector.tensor_copy(sf[:], st[:])
                chunks = []
                for c in range(2):
                    tp = ps_pair(psS, "setup_tp")
                    nc.tensor.transpose(tp[:, 0:nb], sf[:, c * P:(c + 1) * P], ident[:nb, :nb])
                    tf = const.tile([P, nb], F32, name=f"sTf{s}{c}", tag=f"sTf{s}{c}")
                    nc.vector.tensor_copy(tf[:], tp[:, 0:nb])
                    ti = const.tile([P, nb], I32, name=f"sTi{s}{c}", tag=f"sTi{s}{c}")
                    nc.vector.tensor_copy(ti[:], tf[:])
                    chunks.append(ti)
                sT.append(chunks)

            # v accumulators [(128|72), nb] per H-chunk per sentence
            v_all = [[const.tile([m1 - m0, nb], F32, name=f"v{s}{m}", tag=f"v{s}{m}")
                      for m, (m0, m1) in enumerate(HM)] for s in range(2)]

            # ---------------- per-batch loop (1-batch software pipeline) ----
            # H1(b): gather -> eT transposes -> attend MLP -> scores -> exp.
            # H2(b): u1 transpose -> attention sums -> normalize -> compare.
            # Emission order H1(b+1) before H2(b) keeps independent PE work
            # between the engine handoffs of the serial softmax chain.
            def emit_h1(b):
                st = {'b': b}
                lens2 = sm.tile([1, 2 * L], I32, name="lens2", tag="lens2")
                for s in range(2):
                    nc.vector.tensor_copy(lens2[:, s * L:(s + 1) * L],
                                          len_f[s][:, b:b + 1].to_broadcast([1, L]))
                mrow2 = sm.tile([1, 2 * L], F32, name="mrow2", tag="mrow2")
                nc.vector.tensor_tensor(mrow2[:], iota2[:], lens2[:], op=ALU.is_lt)
                lmrow2 = sm.tile([1, 2 * L], BF, name="lmrow2", tag="lmrow2")
                nc.vector.tensor_scalar(lmrow2[:], mrow2[:], 1.0, 30000.0,
                                        op0=ALU.subtract, op1=ALU.mult)
                st['lmrow2'] = lmrow2

                eR = [[], []]
                for s in range(2):
                    for c in range(2):
                        er = gat.tile([P, E1], BF, name=f"eR{s}{c}", tag=f"eR{s}{c}")
                        nc.gpsimd.indirect_dma_start(
                            out=er[:, 0:EMBED], out_offset=None, in_=emb_d[:],
                            in_offset=bass.IndirectOffsetOnAxis(ap=sT[s][c][:, b:b + 1], axis=0),
                        )
                        nc.vector.memset(er[:, EMBED:E1], 1.0)
                        eR[s].append(er)
                e2m = []
                for c in range(2):
                    em = gat.tile([P, E1], BF, name=f"e2m{c}", tag=f"e2m{c}")
                    nc.vector.tensor_tensor(em[:], eR[1][c][:],
                                            mTb[1][c][:, b:b + 1].to_broadcast([P, E1]), op=ALU.mult)
                    e2m.append(em)
                st['eR'] = eR
                st['e2m'] = e2m

                XT = xtp.tile([P, 10 * L], BF, name="XT", tag="XT")
                for s in range(2):
                    nc.vector.memset(XT[0:64, 4 * L + s * L:4 * L + (s + 1) * L], 0.0)
                    tp = ps_tr(f"eTtp{s}")
                    for k, (k0, k1) in enumerate(EK):
                        for c in range(2):
                            nc.tensor.transpose(tp[:k1 - k0, k * L + c * P:k * L + (c + 1) * P],
                                                eR[s][c][:, k0:k1], ident_b[:])
                    for k in range(2):
                        nc.any.tensor_copy(XT[:, 2 * k * L + s * L:2 * k * L + (s + 1) * L],
                                           tp[:, k * L:(k + 1) * L])
                    nc.any.tensor_copy(XT[0:44, 4 * L + s * L:4 * L + (s + 1) * L],
                                       tp[0:44, 2 * L:3 * L])
                st['XT'] = XT

                ha = []
                for m, (m0, m1) in enumerate(HM):
                    pa = ps_pair(psMM, "pa")
                    for k, (k0, k1) in enumerate(EK):
                        nc.tensor.matmul(pa[:m1 - m0, :], W1a_t[k][:, m0:m1],
                                         XT[0:k1 - k0, 2 * k * L:2 * (k + 1) * L],
                                         start=(k == 0), stop=(k == 2))
                    h = hp.tile([m1 - m0, 2 * L], BF, name=f"ha{m}", tag=f"ha{m}")
                    nc.scalar.activation(h[:], pa[:m1 - m0, :], ACTF.Relu,
                                         bias=b1a_t[m][:], scale=1.0)
                    ha.append(h)
                hT = []
                for m, (m0, m1) in enumerate(HM):
                    pb = ps_pair(psMM, "pb")
                    for k2 in range(2):
                        nc.tensor.matmul(pb[:m1 - m0, :], W2a_t[k2][:, m0:m1], ha[k2][:],
                                         start=(k2 == 0), stop=(k2 == 1))
                    h = hp.tile([m1 - m0, 2 * L], BF, name=f"hT{m}", tag=f"hT{m}")
                    nc.scalar.activation(h[:], pb[:m1 - m0, :], ACTF.Relu,
                                         bias=b2a_t[m][:], scale=1.0)
                    hT.append(h)
                st['hT'] = hT

                ep = ps_pair(psS, "score")
                for ic in range(2):
                    for m in range(2):
                        nc.tensor.matmul(ep[:, ic * L:(ic + 1) * L],
                                         hT[m][:, ic * P:(ic + 1) * P], hT[m][:, L:2 * L],
                                         start=(m == 0), stop=(m == 1))
                u0 = sm.tile([P, 2 * L], BF, name="u0", tag="u0")
                for ic in range(2):
                    nc.scalar.activation(u0[:, ic * L:(ic + 1) * L], ep[:, ic * L:(ic + 1) * L],
                                         ACTF.Exp, bias=lmT[0][ic][:, b:b + 1], scale=1.0)
                st['u0'] = u0
                return st

            def emit_h2(st):
                b = st['b']
                eR, e2m, XT, u0 = st['eR'], st['e2m'], st['XT'], st['u0']
                tpu = ps_tr("u1tp")
                for jc in range(2):
                    for ic in range(2):
                        nc.tensor.transpose(tpu[:, jc * L + ic * P:jc * L + (ic + 1) * P],
                                            u0[:, ic * L + jc * P:ic * L + (jc + 1) * P], ident_b[:])
                u1 = sm.tile([P, 2 * L], BF, name="u1", tag="u1")
                nc.any.tensor_copy(u1[:], tpu[:, 0:2 * L])

                for d in range(2):
                    u_t = u0 if d == 0 else u1
                    rhs = eR[0] if d == 0 else e2m
                    s = 1 - d
                    tt = ps_tr(f"bt{d}")
                    for t_ in range(2):
                        q = ps_pair(psA, f"q{d}{t_}")
                        for c in range(2):
                            nc.tensor.matmul(q[:, 0:E1],
                                             u_t[:, c * L + t_ * P:c * L + (t_ + 1) * P],
                                             rhs[c][:, 0:E1], start=(c == 0), stop=(c == 1))
                        dcol = sm.tile([P, 1], F32, name=f"dc{d}{t_}", tag=f"dc{d}{t_}")
                        nc.vector.tensor_scalar(dcol[:], q[:, EMBED:E1], 1e-20, None, op0=ALU.add)
                        rcp = sm.tile([P, 1], F32, name=f"rcp{d}{t_}", tag=f"rcp{d}{t_}")
                        nc.vector.reciprocal(rcp[:], dcol[:])
                        nrm = sm.tile([P, EMBED], BF, name=f"nrm{d}{t_}", tag=f"nrm{d}{t_}")
                        nc.vector.tensor_scalar(nrm[:], q[:, 0:EMBED], rcp[:],
                                                mTf[1 - d][t_][:, b:b + 1],
                                                op0=ALU.mult, op1=ALU.mult)
                        for t, (c0, c1, poff) in enumerate(BCHUNK):
                            nc.tensor.transpose(
                                tt[poff:poff + (c1 - c0), t * L + t_ * P:t * L + (t_ + 1) * P],
                                nrm[:, c0:c1], ident_b[:])
                    nc.any.tensor_copy(XT[64:128, 4 * L + s * L:4 * L + (s + 1) * L],
                                       tt[64:128, 0:L])
                    nc.any.tensor_copy(XT[:, 6 * L + s * L:6 * L + (s + 1) * L],
                                       tt[:, L:2 * L])
                    nc.any.tensor_copy(XT[0:108, 8 * L + s * L:8 * L + (s + 1) * L],
                                       tt[0:108, 2 * L:3 * L])

                r1 = []
                for m, (m0, m1) in enumerate(HM):
                    pc = ps_pair(psMM, "pc")
                    for k in range(5):
                        nc.tensor.matmul(pc[:m1 - m0, :], W1c_t[k][:, m0:m1],
                                         XT[0:XCHUNK[k], 2 * k * L:2 * (k + 1) * L],
                                         start=(k == 0), stop=(k == 4))
                    r = hp.tile([m1 - m0, 2 * L], BF, name=f"r1{m}", tag=f"r1{m}")
                    nc.scalar.activation(r[:], pc[:m1 - m0, :], ACTF.Relu,
                                         bias=b1c_t[m][:], scale=1.0)
                    r1.append(r)
                for m, (m0, m1) in enumerate(HM):
                    pd = ps_pair(psMM, "pd")
                    for k2 in range(2):
                        nc.tensor.matmul(pd[:m1 - m0, :], W2c_t[k2][:, m0:m1], r1[k2][:],
                                         start=(k2 == 0), stop=False)
                    nc.tensor.matmul(pd[:m1 - m0, :], ones_row_b[:, 0:m1 - m0],
                                     st['lmrow2'][:], start=False, stop=True)
                    for s in range(2):
                        scr = hp.tile([m1 - m0, L], BF, name=f"scr{s}{m}", tag=f"scr{s}{m}")
                        nc.scalar.activation(scr[:], pd[:m1 - m0, s * L:(s + 1) * L],
                                             ACTF.Relu, bias=b2c_t[m][:],
                                             scale=1.0, accum_out=v_all[s][m][:, b:b + 1])

                if dbg and b == 0:
                    nc.sync.dma_start(dbg_d['XT'][:], XT[:])
                    nc.sync.dma_start(dbg_d['u0'][:], u0[:])
                    nc.sync.dma_start(dbg_d['u1'][:], u1[:])
                    nc.sync.dma_start(dbg_d['hT0'][:], st['hT'][0][:])

            prev = emit_h1(0)
            for b in range(1, nb):
                cur = emit_h1(b)
                emit_h2(prev)
                prev = cur
            emit_h2(prev)

            # ---------------- aggregate ----------------
            if dbg:
                nc.sync.dma_start(dbg_d['v00'][:], v_all[0][0][:])
            vb = []
            for s in range(2):
                for m, (m0, m1) in enumerate(HM):
                    t = const.tile([m1 - m0, nb], BF, name=f"vb{s}{m}", tag=f"vb{s}{m}")
                    nc.vector.tensor_copy(t[:], v_all[s][m][:])
                    vb.append(t)
            g1 = []
            gp = ps_pair(psMM, "pa")
            for m, (m0, m1) in enumerate(HM):
                for k in range(4):
                    nc.tensor.matmul(gp[:m1 - m0, m * nb:(m + 1) * nb],
                                     W1g_t[k][:, m0:m1], vb[k][:],
                                     start=(k == 0), stop=(k == 3))
            for m, (m0, m1) in enumerate(HM):
                g = const.tile([m1 - m0, nb], BF, name=f"g1{m}", tag=f"g1{m}")
                nc.scalar.activation(g[:], gp[:m1 - m0, m * nb:(m + 1) * nb],
                                     ACTF.Relu, bias=b1g_t[m][:], scale=1.0)
                g1.append(g)
            op = ps_pair(psMM, "pb")
            for k2 in range(2):
                nc.tensor.matmul(op[0:2, 0:nb], W2g_t[k2][:], g1[k2][:],
                                 start=(k2 == 0), stop=(k2 == 1))
            osb = const.tile([2, nb], F32, name="osb", tag="osb")
            nc.scalar.activation(osb[:], op[0:2, 0:nb], ACTF.Identity, bias=b2g_t[:], scale=1.0)
            nc.sync.dma_start(out_d[:].rearrange("b o -> o b"), osb[:])

    nc.compile()
    return nc


def _shard_inputs(inputs, nb=BC, ncores=NCORES):
    import ml_dtypes
    bf16 = ml_dtypes.bfloat16
    f = np.ascontiguousarray

    emb_bf = f(inputs['emb'].astype(bf16))
    # W1c packed into 5 chunks of 128 rows: [0:256] e-rows, chunk2 = 44 e-tail
    # rows + 20 zero rows + 64 beta rows, then beta rows 64:192, 192:300.
    W1c = inputs['W1c'].astype(np.float32)
    W1c_p = np.zeros((640, HIDDEN), np.float32)
    W1c_p[0:256] = W1c[0:256]
    W1c_p[256:300] = W1c[256:300]
    W1c_p[320:384] = W1c[300:364]
    W1c_p[384:512] = W1c[364:492]
    W1c_p[512:620] = W1c[492:600]
    wb = {k: f(inputs[k].astype(bf16)) for k in ('W1a', 'W2a', 'W2c', 'W1g', 'W2g')}
    wb['W1c'] = f(W1c_p.astype(bf16))

    maps = []
    for c in range(ncores):
        sl = slice(c * nb, (c + 1) * nb)
        maps.append(dict(
            emb=emb_bf,
            s1=f(inputs['s1'][sl].astype(np.int32)),
            s2=f(inputs['s2'][sl].astype(np.int32)),
            len1=f(inputs['len1'][sl].reshape(nb, 1).astype(np.int32)),
            len2=f(inputs['len2'][sl].reshape(nb, 1).astype(np.int32)),
            b1a=f(inputs['b1a'].reshape(-1, 1).astype(np.float32)),
            b2a=f(inputs['b2a'].reshape(-1, 1).astype(np.float32)),
            b1c=f(inputs['b1c'].reshape(-1, 1).astype(np.float32)),
            b2c=f(inputs['b2c'].reshape(-1, 1).astype(np.float32)),
            b1g=f(inputs['b1g'].reshape(-1, 1).astype(np.float32)),
            b2g=f(inputs['b2g'].reshape(-1, 1).astype(np.float32)),
            **wb,
        ))
    return maps


def kernel(**inputs):
    from concourse.bass_utils import run_bass_kernel_spmd
    if 'prog' not in _prog_cache:
        _prog_cache['prog'] = build_program(BC)
    nc = _prog_cache['prog']
    in_maps = _shard_inputs(inputs)
    res = run_bass_kernel_spmd(nc, in_maps, core_ids=list(range(NCORES)))
    out = np.concatenate([res.results[c]["out"] for c in range(NCORES)], axis=0)
    return out.astype(np.float32)


# revision 23
# speedup vs baseline: 4.0897x; 1.0187x over previous
"""Trainium2 Bass kernel for DecomposableAttention (B=512, L=256, V=50000, E=300, H=200).

Data-parallel over batch across 8 cores (64 batches/core).  All matmuls bf16
(1 cycle/row), fp32 PSUM.  Per batch:

- indirect-DMA gather of bf16 embedding rows (table converted host-side) with a
  ones/mask column appended for the softmax denominators
- PE transposes to the E-on-partitions layout; both sentences packed side by
  side in one [128, 2560] x tile so every MLP matmul/activation runs once with
  a 512-wide free dim
- attend MLP -> scores (one direction) -> exp without the max stabilizer
  (scores bounded ~11; the stabilizer cancels exactly after normalization);
  the reverse-direction exp'd scores come from a PE transpose of u0
- attention sums in [token, 301] layout: the denominator lands in column 300 of
  the same matmul, so 1/(den+eps) is a [128,1] per-partition op and the
  normalization + output masking fuse into one tensor_scalar; the result is
  PE-transposed into the compare layout
- compare MLP over 5 packed 128-row K-chunks; masked column-sum fused via a
  K=1 logmask matmul into PSUM + the activation engine's accum_out

PSUM (8 banks x 2KB): psT 2 (bf16 transpose staging), psMM 2 (MLP pairs),
psS 2 (scores), psA 2 (attention sums).  Accumulation groups never interleave
within a bank (hardware breaks interleaved groups) and tiles never straddle
bank boundaries.
"""
import sys

if '/opt/trn_rl_repo' not in sys.path:
    sys.path.insert(0, '/opt/trn_rl_repo')

import numpy as np

B, L, VOCAB, EMBED, HIDDEN = 512, 256, 50000, 300, 200
NCORES = 8
BC = B // NCORES  # batches per core
P = 128
E1 = EMBED + 1    # emb cols + ones/mask column

_prog_cache = {}

# K-chunk layout of the 600-dim compare input [e (300); beta (300)].
# Chunk 2 mixes the e tail (44 rows) + 20 zero rows + beta[0:64] at partition
# offset 64; W1c is host-packed to match.
XCHUNK = [128, 128, 128, 128, 108]
BCHUNK = [(0, 64, 64), (64, 192, 0), (192, 300, 0)]  # beta col ranges + partition offset


def build_program(nb=BC, ndev=NCORES, dbg=False):
    import concourse.bass as bass
    import concourse.bacc as bacc
    import concourse.tile as tile
    import concourse.mybir as mybir
    from concourse.masks import make_identity

    F32 = mybir.dt.float32
    BF = mybir.dt.bfloat16
    I32 = mybir.dt.int32
    ALU = mybir.AluOpType
    ACTF = mybir.ActivationFunctionType
    EK = [(0, 128), (128, 256), (256, 300)]   # E contraction chunks
    HM = [(0, 128), (128, 200)]               # H chunks (128, 72)

    nc = bacc.Bacc("TRN2", num_devices=ndev)

    emb_d = nc.dram_tensor("emb", [VOCAB, EMBED], BF, kind="ExternalInput")
    s1_d = nc.dram_tensor("s1", [nb, L], I32, kind="ExternalInput")
    s2_d = nc.dram_tensor("s2", [nb, L], I32, kind="ExternalInput")
    len1_d = nc.dram_tensor("len1", [nb, 1], I32, kind="ExternalInput")
    len2_d = nc.dram_tensor("len2", [nb, 1], I32, kind="ExternalInput")
    W1a_d = nc.dram_tensor("W1a", [EMBED, HIDDEN], BF, kind="ExternalInput")
    W2a_d = nc.dram_tensor("W2a", [HIDDEN, HIDDEN], BF, kind="ExternalInput")
    W1c_d = nc.dram_tensor("W1c", [640, HIDDEN], BF, kind="ExternalInput")   # host-packed 5x128
    W2c_d = nc.dram_tensor("W2c", [HIDDEN, HIDDEN], BF, kind="ExternalInput")
    W1g_d = nc.dram_tensor("W1g", [2 * HIDDEN, HIDDEN], BF, kind="ExternalInput")
    W2g_d = nc.dram_tensor("W2g", [HIDDEN, 2], BF, kind="ExternalInput")
    b1a_d = nc.dram_tensor("b1a", [HIDDEN, 1], F32, kind="ExternalInput")
    b2a_d = nc.dram_tensor("b2a", [HIDDEN, 1], F32, kind="ExternalInput")
    b1c_d = nc.dram_tensor("b1c", [HIDDEN, 1], F32, kind="ExternalInput")
    b2c_d = nc.dram_tensor("b2c", [HIDDEN, 1], F32, kind="ExternalInput")
    b1g_d = nc.dram_tensor("b1g", [HIDDEN, 1], F32, kind="ExternalInput")
    b2g_d = nc.dram_tensor("b2g", [2, 1], F32, kind="ExternalInput")
    out_d = nc.dram_tensor("out", [nb, 2], F32, kind="ExternalOutput")
    if dbg:
        dbg_d = {
            'XT': nc.dram_tensor("dbg_XT", [P, 10 * L], BF, kind="ExternalOutput"),
            'u0': nc.dram_tensor("dbg_u0", [P, 2 * L], BF, kind="ExternalOutput"),
            'u1': nc.dram_tensor("dbg_u1", [P, 2 * L], BF, kind="ExternalOutput"),
            'hT0': nc.dram_tensor("dbg_hT0", [P, 2 * L], BF, kind="ExternalOutput"),
            'v00': nc.dram_tensor("dbg_v00", [P, nb], F32, kind="ExternalOutput"),
        }

    with tile.TileContext(nc) as tc:
        import contextlib
        ctx = contextlib.ExitStack()
        with ctx:
            const = ctx.enter_context(tc.tile_pool(name="const", bufs=1))
            psT = ctx.enter_context(tc.tile_pool(name="psT", bufs=2, space="PSUM"))
            psMM = ctx.enter_context(tc.tile_pool(name="psMM", bufs=3, space="PSUM"))
            psS = ctx.enter_context(tc.tile_pool(name="psS", bufs=1, space="PSUM"))
            psA = ctx.enter_context(tc.tile_pool(name="psA", bufs=2, space="PSUM"))
            gat = ctx.enter_context(tc.tile_pool(name="gat", bufs=3))
            xtp = ctx.enter_context(tc.tile_pool(name="xtp", bufs=2))
            hp = ctx.enter_context(tc.tile_pool(name="hp", bufs=3))
            sm = ctx.enter_context(tc.tile_pool(name="sm", bufs=3))

            def ps_pair(pool, nm):
                return pool.tile([P, 512], F32, name=nm, tag=pool.name)

            def ps_tr(nm):
                # full 2KB so every pool buffer is PSUM-bank aligned
                return psT.tile([P, 1024], BF, name=nm, tag="psT")

            # ---------------- constants ----------------
            ident = const.tile([P, P], F32)
            make_identity(nc, ident[:])
            ident_b = const.tile([P, P], BF)
            nc.vector.tensor_copy(ident_b[:], ident[:])
            ones_row_b = const.tile([1, P], BF)
            nc.vector.memset(ones_row_b[:], 1.0)

            # weights (bf16, direct DMA)
            W1a_t = [const.tile([k1 - k0, HIDDEN], BF, name=f"W1a{i}", tag=f"W1a{i}")
                     for i, (k0, k1) in enumerate(EK)]
            for i, (k0, k1) in enumerate(EK):
                nc.sync.dma_start(W1a_t[i][:], W1a_d[k0:k1, :])
            W2a_t = [const.tile([m1 - m0, HIDDEN], BF, name=f"W2a{i}", tag=f"W2a{i}")
                     for i, (m0, m1) in enumerate(HM)]
            for i, (m0, m1) in enumerate(HM):
                nc.sync.dma_start(W2a_t[i][:], W2a_d[m0:m1, :])
            W1c_t = [const.tile([k, HIDDEN], BF, name=f"W1c{i}", tag=f"W1c{i}")
                     for i, k in enumerate(XCHUNK)]
            for i in range(5):
                nc.sync.dma_start(W1c_t[i][:], W1c_d[i * 128:i * 128 + XCHUNK[i], :])
            W2c_t = [const.tile([m1 - m0, HIDDEN], BF, name=f"W2c{i}", tag=f"W2c{i}")
                     for i, (m0, m1) in enumerate(HM)]
            for i, (m0, m1) in enumerate(HM):
                nc.sync.dma_start(W2c_t[i][:], W2c_d[m0:m1, :])
            GK = [(0, 128), (128, 200), (200, 328), (328, 400)]
            W1g_t = [const.tile([k1 - k0, HIDDEN], BF, name=f"W1g{i}", tag=f"W1g{i}")
                     for i, (k0, k1) in enumerate(GK)]
            for i, (k0, k1) in enumerate(GK):
                nc.sync.dma_start(W1g_t[i][:], W1g_d[k0:k1, :])
            W2g_t = [const.tile([m1 - m0, 2], BF, name=f"W2g{i}", tag=f"W2g{i}")
                     for i, (m0, m1) in enumerate(HM)]
            for i, (m0, m1) in enumerate(HM):
                nc.sync.dma_start(W2g_t[i][:], W2g_d[m0:m1, :])

            def bias2(d, nm):
                t = [const.tile([m1 - m0, 1], F32, name=f"b{nm}{i}", tag=f"b{nm}{i}")
                     for i, (m0, m1) in enumerate(HM)]
                for i, (m0, m1) in enumerate(HM):
                    nc.sync.dma_start(t[i][:], d[m0:m1, :])
                return t
            b1a_t, b2a_t = bias2(b1a_d, "1a"), bias2(b2a_d, "2a")
            b1c_t, b2c_t = bias2(b1c_d, "1c"), bias2(b2c_d, "2c")
            b1g_t = bias2(b1g_d, "1g")
            b2g_t = const.tile([2, 1], F32)
            nc.sync.dma_start(b2g_t[:], b2g_d[:])

            # masks / lengths
            len_t = []
            len_f = []
            for s, ld in ((0, len1_d), (1, len2_d)):
                lt = const.tile([nb, 1], I32, name=f"len{s}", tag=f"len{s}")
                nc.sync.dma_start(lt[:], ld[:])
                len_t.append(lt)
                lf = const.tile([1, nb], I32, name=f"lenf{s}", tag=f"lenf{s}")
                nc.sync.dma_start(lf[:], ld[:].rearrange("n one -> one n"))
                len_f.append(lf)
            iota_t = const.tile([nb, L], I32)
            nc.gpsimd.iota(iota_t[:], pattern=[[1, L]], base=0, channel_multiplier=0)
            iota2 = const.tile([1, 2 * L], I32)   # 0..255 twice
            nc.gpsimd.iota(iota2[:], pattern=[[0, 2], [1, L]], base=0, channel_multiplier=0)

            lmT = []   # transposed logmask1 [128, nb] f32, 2 chunks (exp bias)
            mTb = []   # transposed 0/1 masks [128, nb] bf16, 2 chunks per sentence
            mTf = []   # same masks in f32 (tensor_scalar scalar2 operand)
            for s in range(2):
                m = const.tile([nb, L], F32, name=f"mask{s}", tag=f"mask{s}")
                nc.vector.tensor_tensor(m[:], iota_t[:], len_t[s][:].to_broadcast([nb, L]), op=ALU.is_lt)
                lts, mts, mfs = [], [], []
                for c in range(2):
                    if s == 0:
                        lm = const.tile([nb, L], F32, name="lm0", tag="lm0")
                        nc.vector.tensor_scalar(lm[:], m[:], 1.0, 30000.0,
                                                op0=ALU.subtract, op1=ALU.mult)
                        tp = ps_pair(psS, "setup_tp")
                        nc.tensor.transpose(tp[:, 0:nb], lm[:, c * P:(c + 1) * P], ident[:nb, :nb])
                        lt = const.tile([P, nb], F32, name=f"lmT{s}{c}", tag=f"lmT{s}{c}")
                        nc.vector.tensor_copy(lt[:], tp[:, 0:nb])
                        lts.append(lt)
                    tp2 = ps_pair(psS, "setup_tp")
                    nc.tensor.transpose(tp2[:, 0:nb], m[:, c * P:(c + 1) * P], ident[:nb, :nb])
                    mt = const.tile([P, nb], BF, name=f"mTb{s}{c}", tag=f"mTb{s}{c}")
                    nc.vector.tensor_copy(mt[:], tp2[:, 0:nb])
                    mts.append(mt)
                    mf = const.tile([P, nb], F32, name=f"mTf{s}{c}", tag=f"mTf{s}{c}")
                    nc.vector.tensor_copy(mf[:], tp2[:, 0:nb])
                    mfs.append(mf)
                lmT.append(lts)
                mTb.append(mts)
                mTf.append(mfs)

            # token indices, transposed to [128, nb] int32 per chunk
            sT = []
            for s, sd in ((0, s1_d), (1, s2_d)):
                st = const.tile([nb, L], I32, name=f"s{s}", tag=f"s{s}")
                nc.sync.dma_start(st[:], sd[:])
                sf = const.tile([nb, L], F32, name=f"sf{s}", tag=f"sf{s}")
                nc.vector.tensor_copy(sf[:], st[:])
                chunks = []
                for c in range(2):
                    tp = ps_pair(psS, "setup_tp")
                    nc.tensor.transpose(tp[:, 0:nb], sf[:, c * P:(c + 1) * P], ident[:nb, :nb])
                    tf = const.tile([P, nb], F32, name=f"sTf{s}{c}", tag=f"sTf{s}{c}")
                    nc.vector.tensor_copy(tf[:], tp[:, 0:nb])
                    ti = const.tile([P, nb], I32, name=f"sTi{s}{c}", tag=f"sTi{s}{c}")
                    nc.vector.tensor_copy(ti[:], tf[:])
                    chunks.append(ti)
                sT.append(chunks)

            # v accumulators [(128|72), nb] per H-chunk per sentence
            v_all = [[const.tile([m1 - m0, nb], F32, name=f"v{s}{m}", tag=f"v{s}{m}")
                      for m, (m0, m1) in enumerate(HM)] for s in range(2)]

            # ---------------- per-batch loop (1-batch software pipeline) ----
            # H1(b): gather -> eT transposes -> attend MLP -> scores -> exp.
            # H2(b): u1 transpose -> attention sums -> normalize -> compare.
            # Emission order H1(b+1) before H2(b) keeps independent PE work
            # between the engine handoffs of the serial softmax chain.
            def emit_h1(b):
                st = {'b': b}
                lens2 = sm.tile([1, 2 * L], I32, name="lens2", tag="lens2")
                for s in range(2):
                    nc.vector.tensor_copy(lens2[:, s * L:(s + 1) * L],
                                          len_f[s][:, b:b + 1].to_broadcast([1, L]))
                mrow2 = sm.tile([1, 2 * L], F32, name="mrow2", tag="mrow2")
                nc.vector.tensor_tensor(mrow2[:], iota2[:], lens2[:], op=ALU.is_lt)
                lmrow2 = sm.tile([1, 2 * L], BF, name="lmrow2", tag="lmrow2")
                nc.vector.tensor_scalar(lmrow2[:], mrow2[:], 1.0, 30000.0,
                                        op0=ALU.subtract, op1=ALU.mult)
                st['lmrow2'] = lmrow2

                eR = [[], []]
                for s in range(2):
                    for c in range(2):
                        er = gat.tile([P, E1], BF, name=f"eR{s}{c}", tag=f"eR{s}{c}")
                        nc.gpsimd.indirect_dma_start(
                            out=er[:, 0:EMBED], out_offset=None, in_=emb_d[:],
                            in_offset=bass.IndirectOffsetOnAxis(ap=sT[s][c][:, b:b + 1], axis=0),
                        )
                        if b < 3:
                            nc.vector.memset(er[:, EMBED:E1], 1.0)
                        eR[s].append(er)
                e2m = []
                for c in range(2):
                    em = gat.tile([P, E1], BF, name=f"e2m{c}", tag=f"e2m{c}")
                    nc.gpsimd.tensor_tensor(em[:], eR[1][c][:],
                                            mTb[1][c][:, b:b + 1].to_broadcast([P, E1]), op=ALU.mult)
                    e2m.append(em)
                st['eR'] = eR
                st['e2m'] = e2m

                XT = xtp.tile([P, 10 * L], BF, name="XT", tag="XT")
                for s in range(2):
                    if b < 2:
                        nc.vector.memset(XT[0:64, 4 * L + s * L:4 * L + (s + 1) * L], 0.0)
                    tp = ps_tr(f"eTtp{s}")
                    for k, (k0, k1) in enumerate(EK):
                        for c in range(2):
                            nc.tensor.transpose(tp[:k1 - k0, k * L + c * P:k * L + (c + 1) * P],
                                                eR[s][c][:, k0:k1], ident_b[:])
                    for k in range(2):
                        nc.any.tensor_copy(XT[:, 2 * k * L + s * L:2 * k * L + (s + 1) * L],
                                           tp[:, k * L:(k + 1) * L])
                    nc.any.tensor_copy(XT[0:44, 4 * L + s * L:4 * L + (s + 1) * L],
                                       tp[0:44, 2 * L:3 * L])
                st['XT'] = XT

                ha = []
                for m, (m0, m1) in enumerate(HM):
                    pa = ps_pair(psMM, "pa")
                    for k, (k0, k1) in enumerate(EK):
                        nc.tensor.matmul(pa[:m1 - m0, :], W1a_t[k][:, m0:m1],
                                         XT[0:k1 - k0, 2 * k * L:2 * (k + 1) * L],
                                         start=(k == 0), stop=(k == 2))
                    h = hp.tile([m1 - m0, 2 * L], BF, name=f"ha{m}", tag=f"ha{m}")
                    nc.scalar.activation(h[:], pa[:m1 - m0, :], ACTF.Relu,
                                         bias=b1a_t[m][:], scale=1.0)
                    ha.append(h)
                hT = []
                for m, (m0, m1) in enumerate(HM):
                    pb = ps_pair(psMM, "pb")
                    for k2 in range(2):
                        nc.tensor.matmul(pb[:m1 - m0, :], W2a_t[k2][:, m0:m1], ha[k2][:],
                                         start=(k2 == 0), stop=(k2 == 1))
                    h = hp.tile([m1 - m0, 2 * L], BF, name=f"hT{m}", tag=f"hT{m}")
                    nc.scalar.activation(h[:], pb[:m1 - m0, :], ACTF.Relu,
                                         bias=b2a_t[m][:], scale=1.0)
                    hT.append(h)
                st['hT'] = hT

                ep = ps_pair(psS, "score")
                for ic in range(2):
                    for m in range(2):
                        nc.tensor.matmul(ep[:, ic * L:(ic + 1) * L],
                                         hT[m][:, ic * P:(ic + 1) * P], hT[m][:, L:2 * L],
                                         start=(m == 0), stop=(m == 1))
                u0 = sm.tile([P, 2 * L], BF, name="u0", tag="u0")
                for ic in range(2):
                    nc.scalar.activation(u0[:, ic * L:(ic + 1) * L], ep[:, ic * L:(ic + 1) * L],
                                         ACTF.Exp, bias=lmT[0][ic][:, b:b + 1], scale=1.0)
                st['u0'] = u0
                return st

            def emit_h2(st):
                b = st['b']
                eR, e2m, XT, u0 = st['eR'], st['e2m'], st['XT'], st['u0']
                tpu = ps_tr("u1tp")
                for jc in range(2):
                    for ic in range(2):
                        nc.tensor.transpose(tpu[:, jc * L + ic * P:jc * L + (ic + 1) * P],
                                            u0[:, ic * L + jc * P:ic * L + (jc + 1) * P], ident_b[:])
                u1 = sm.tile([P, 2 * L], BF, name="u1", tag="u1")
                nc.any.tensor_copy(u1[:], tpu[:, 0:2 * L])

                for d in range(2):
                    u_t = u0 if d == 0 else u1
                    rhs = eR[0] if d == 0 else e2m
                    s = 1 - d
                    tt = ps_tr(f"bt{d}")
                    for t_ in range(2):
                        q = ps_pair(psA, f"q{d}{t_}")
                        for c in range(2):
                            nc.tensor.matmul(q[:, 0:E1],
                                             u_t[:, c * L + t_ * P:c * L + (t_ + 1) * P],
                                             rhs[c][:, 0:E1], start=(c == 0), stop=(c == 1))
                        dcol = sm.tile([P, 1], F32, name=f"dc{d}{t_}", tag=f"dc{d}{t_}")
                        nc.vector.tensor_scalar(dcol[:], q[:, EMBED:E1], 1e-20, None, op0=ALU.add)
                        rcp = sm.tile([P, 1], F32, name=f"rcp{d}{t_}", tag=f"rcp{d}{t_}")
                        nc.vector.reciprocal(rcp[:], dcol[:])
                        nrm = sm.tile([P, EMBED], BF, name=f"nrm{d}{t_}", tag=f"nrm{d}{t_}")
                        nc.vector.tensor_scalar(nrm[:], q[:, 0:EMBED], rcp[:],
                                                mTf[1 - d][t_][:, b:b + 1],
                                                op0=ALU.mult, op1=ALU.mult)
                        for t, (c0, c1, poff) in enumerate(BCHUNK):
                            nc.tensor.transpose(
                                tt[poff:poff + (c1 - c0), t * L + t_ * P:t * L + (t_ + 1) * P],
                                nrm[:, c0:c1], ident_b[:])
                    nc.any.tensor_copy(XT[64:128, 4 * L + s * L:4 * L + (s + 1) * L],
                                       tt[64:128, 0:L])
                    nc.any.tensor_copy(XT[:, 6 * L + s * L:6 * L + (s + 1) * L],
                                       tt[:, L:2 * L])
                    nc.any.tensor_copy(XT[0:108, 8 * L + s * L:8 * L + (s + 1) * L],
                                       tt[0:108, 2 * L:3 * L])

                r1 = []
                for m, (m0, m1) in enumerate(HM):
                    pc = ps_pair(psMM, "pc")
                    for k in range(5):
                        nc.tensor.matmul(pc[:m1 - m0, :], W1c_t[k][:, m0:m1],
                                         XT[0:XCHUNK[k], 2 * k * L:2 * (k + 1) * L],
                                         start=(k == 0), stop=(k == 4))
                    r = hp.tile([m1 - m0, 2 * L], BF, name=f"r1{m}", tag=f"r1{m}")
                    nc.scalar.activation(r[:], pc[:m1 - m0, :], ACTF.Relu,
                                         bias=b1c_t[m][:], scale=1.0)
                    r1.append(r)
                for m, (m0, m1) in enumerate(HM):
                    pd = ps_pair(psMM, "pd")
                    for k2 in range(2):
                        nc.tensor.matmul(pd[:m1 - m0, :], W2c_t[k2][:, m0:m1], r1[k2][:],
                                         start=(k2 == 0), stop=False)
                    nc.tensor.matmul(pd[:m1 - m0, :], ones_row_b[:, 0:m1 - m0],
                                     st['lmrow2'][:], start=False, stop=True)
                    for s in range(2):
                        scr = hp.tile([m1 - m0, L], BF, name=f"scr{s}{m}", tag=f"scr{s}{m}")
                        nc.scalar.activation(scr[:], pd[:m1 - m0, s * L:(s + 1) * L],
                                             ACTF.Relu, bias=b2c_t[m][:],
                                             scale=1.0, accum_out=v_all[s][m][:, b:b + 1])

                if dbg and b == 0:
                    nc.sync.dma_start(dbg_d['XT'][:], XT[:])
                    nc.sync.dma_start(dbg_d['u0'][:], u0[:])
                    nc.sync.dma_start(dbg_d['u1'][:], u1[:])
                    nc.sync.dma_start(dbg_d['hT0'][:], st['hT'][0][:])

            prev = emit_h1(0)
            for b in range(1, nb):
                cur = emit_h1(b)
                emit_h2(prev)
                prev = cur
            emit_h2(prev)

            # ---------------- aggregate ----------------
            if dbg:
                nc.sync.dma_start(dbg_d['v00'][:], v_all[0][0][:])
            vb = []
            for s in range(2):
                for m, (m0, m1) in enumerate(HM):
                    t = const.tile([m1 - m0, nb], BF, name=f"vb{s}{m}", tag=f"vb{s}{m}")
                    nc.vector.tensor_copy(t[:], v_all[s][m][:])
                    vb.append(t)
            g1 = []
            gp = ps_pair(psMM, "pa")
            for m, (m0, m1) in enumerate(HM):
                for k in range(4):
                    nc.tensor.matmul(gp[:m1 - m0, m * nb:(m + 1) * nb],
                                     W1g_t[k][:, m0:m1], vb[k][:],
                                     start=(k == 0), stop=(k == 3))
            for m, (m0, m1) in enumerate(HM):
                g = const.tile([m1 - m0, nb], BF, name=f"g1{m}", tag=f"g1{m}")
                nc.scalar.activation(g[:], gp[:m1 - m0, m * nb:(m + 1) * nb],
                                     ACTF.Relu, bias=b1g_t[m][:], scale=1.0)
                g1.append(g)
            op = ps_pair(psMM, "pb")
            for k2 in range(2):
                nc.tensor.matmul(op[0:2, 0:nb], W2g_t[k2][:], g1[k2][:],
                                 start=(k2 == 0), stop=(k2 == 1))
            osb = const.tile([2, nb], F32, name="osb", tag="osb")
            nc.scalar.activation(osb[:], op[0:2, 0:nb], ACTF.Identity, bias=b2g_t[:], scale=1.0)
            nc.sync.dma_start(out_d[:].rearrange("b o -> o b"), osb[:])

    nc.compile()
    return nc


def _shard_inputs(inputs, nb=BC, ncores=NCORES):
    import ml_dtypes
    bf16 = ml_dtypes.bfloat16
    f = np.ascontiguousarray

    emb_bf = f(inputs['emb'].astype(bf16))
    # W1c packed into 5 chunks of 128 rows: [0:256] e-rows, chunk2 = 44 e-tail
    # rows + 20 zero rows + 64 beta rows, then beta rows 64:192, 192:300.
    W1c = inputs['W1c'].astype(np.float32)
    W1c_p = np.zeros((640, HIDDEN), np.float32)
    W1c_p[0:256] = W1c[0:256]
    W1c_p[256:300] = W1c[256:300]
    W1c_p[320:384] = W1c[300:364]
    W1c_p[384:512] = W1c[364:492]
    W1c_p[512:620] = W1c[492:600]
    wb = {k: f(inputs[k].astype(bf16)) for k in ('W1a', 'W2a', 'W2c', 'W1g', 'W2g')}
    wb['W1c'] = f(W1c_p.astype(bf16))

    maps = []
    for c in range(ncores):
        sl = slice(c * nb, (c + 1) * nb)
        maps.append(dict(
            emb=emb_bf,
            s1=f(inputs['s1'][sl].astype(np.int32)),
            s2=f(inputs['s2'][sl].astype(np.int32)),
            len1=f(inputs['len1'][sl].reshape(nb, 1).astype(np.int32)),
            len2=f(inputs['len2'][sl].reshape(nb, 1).astype(np.int32)),
            b1a=f(inputs['b1a'].reshape(-1, 1).astype(np.float32)),
            b2a=f(inputs['b2a'].reshape(-1, 1).astype(np.float32)),
            b1c=f(inputs['b1c'].reshape(-1, 1).astype(np.float32)),
            b2c=f(inputs['b2c'].reshape(-1, 1).astype(np.float32)),
            b1g=f(inputs['b1g'].reshape(-1, 1).astype(np.float32)),
            b2g=f(inputs['b2g'].reshape(-1, 1).astype(np.float32)),
            **wb,
        ))
    return maps


def kernel(**inputs):
    from concourse.bass_utils import run_bass_kernel_spmd
    if 'prog' not in _prog_cache:
        _prog_cache['prog'] = build_program(BC)
    nc = _prog_cache['prog']
    in_maps = _shard_inputs(inputs)
    res = run_bass_kernel_spmd(nc, in_maps, core_ids=list(range(NCORES)))
    out = np.concatenate([res.results[c]["out"] for c in range(NCORES)], axis=0)
    return out.astype(np.float32)


# revision 24
# speedup vs baseline: 4.1154x; 1.0063x over previous
"""Trainium2 Bass kernel for DecomposableAttention (B=512, L=256, V=50000, E=300, H=200).

Data-parallel over batch across 8 cores (64 batches/core).  All matmuls bf16
(1 cycle/row), fp32 PSUM.  Per batch:

- indirect-DMA gather of bf16 embedding rows (table converted host-side) with a
  ones/mask column appended for the softmax denominators
- PE transposes to the E-on-partitions layout; both sentences packed side by
  side in one [128, 2560] x tile so every MLP matmul/activation runs once with
  a 512-wide free dim
- attend MLP -> scores (one direction) -> exp without the max stabilizer
  (scores bounded ~11; the stabilizer cancels exactly after normalization);
  the reverse-direction exp'd scores come from a PE transpose of u0
- attention sums in [token, 301] layout: the denominator lands in column 300 of
  the same matmul, so 1/(den+eps) is a [128,1] per-partition op and the
  normalization + output masking fuse into one tensor_scalar; the result is
  PE-transposed into the compare layout
- compare MLP over 5 packed 128-row K-chunks; masked column-sum fused via a
  K=1 logmask matmul into PSUM + the activation engine's accum_out

PSUM (8 banks x 2KB): psT 2 (bf16 transpose staging), psMM 2 (MLP pairs),
psS 2 (scores), psA 2 (attention sums).  Accumulation groups never interleave
within a bank (hardware breaks interleaved groups) and tiles never straddle
bank boundaries.
"""
import sys

if '/opt/trn_rl_repo' not in sys.path:
    sys.path.insert(0, '/opt/trn_rl_repo')

import numpy as np

B, L, VOCAB, EMBED, HIDDEN = 512, 256, 50000, 300, 200
NCORES = 8
BC = B // NCORES  # batches per core
P = 128
E1 = EMBED + 1    # emb cols + ones/mask column

_prog_cache = {}

# K-chunk layout of the 600-dim compare input [e (300); beta (300)].
# Chunk 2 mixes the e tail (44 rows) + 20 zero rows + beta[0:64] at partition
# offset 64; W1c is host-packed to match.
XCHUNK = [128, 128, 128, 128, 108]
BCHUNK = [(0, 64, 64), (64, 192, 0), (192, 300, 0)]  # beta col ranges + partition offset


def build_program(nb=BC, ndev=NCORES, dbg=False):
    import concourse.bass as bass
    import concourse.bacc as bacc
    import concourse.tile as tile
    import concourse.mybir as mybir
    from concourse.masks import make_identity

    F32 = mybir.dt.float32
    BF = mybir.dt.bfloat16
    I32 = mybir.dt.int32
    ALU = mybir.AluOpType
    ACTF = mybir.ActivationFunctionType
    EK = [(0, 128), (128, 256), (256, 300)]   # E contraction chunks
    HM = [(0, 128), (128, 256)]               # H chunks, zero-padded to 2x128
    HP = 256                                  # padded hidden dim

    nc = bacc.Bacc("TRN2", num_devices=ndev)

    emb_d = nc.dram_tensor("emb", [VOCAB, EMBED], BF, kind="ExternalInput")
    s1_d = nc.dram_tensor("s1", [nb, L], I32, kind="ExternalInput")
    s2_d = nc.dram_tensor("s2", [nb, L], I32, kind="ExternalInput")
    len1_d = nc.dram_tensor("len1", [nb, 1], I32, kind="ExternalInput")
    len2_d = nc.dram_tensor("len2", [nb, 1], I32, kind="ExternalInput")
    W1a_d = nc.dram_tensor("W1a", [EMBED, 256], BF, kind="ExternalInput")
    W2a_d = nc.dram_tensor("W2a", [256, 256], BF, kind="ExternalInput")
    W1c_d = nc.dram_tensor("W1c", [640, 256], BF, kind="ExternalInput")   # host-packed 5x128
    W2c_d = nc.dram_tensor("W2c", [256, 256], BF, kind="ExternalInput")
    W1g_d = nc.dram_tensor("W1g", [512, 256], BF, kind="ExternalInput")
    W2g_d = nc.dram_tensor("W2g", [256, 2], BF, kind="ExternalInput")
    b1a_d = nc.dram_tensor("b1a", [256, 1], F32, kind="ExternalInput")
    b2a_d = nc.dram_tensor("b2a", [256, 1], F32, kind="ExternalInput")
    b1c_d = nc.dram_tensor("b1c", [256, 1], F32, kind="ExternalInput")
    b2c_d = nc.dram_tensor("b2c", [256, 1], F32, kind="ExternalInput")
    b1g_d = nc.dram_tensor("b1g", [256, 1], F32, kind="ExternalInput")
    b2g_d = nc.dram_tensor("b2g", [2, 1], F32, kind="ExternalInput")
    out_d = nc.dram_tensor("out", [nb, 2], F32, kind="ExternalOutput")
    if dbg:
        dbg_d = {
            'XT': nc.dram_tensor("dbg_XT", [P, 10 * L], BF, kind="ExternalOutput"),
            'u0': nc.dram_tensor("dbg_u0", [P, 2 * L], BF, kind="ExternalOutput"),
            'u1': nc.dram_tensor("dbg_u1", [P, 2 * L], BF, kind="ExternalOutput"),
            'hT0': nc.dram_tensor("dbg_hT0", [P, 2 * L], BF, kind="ExternalOutput"),
            'v00': nc.dram_tensor("dbg_v00", [P, nb], F32, kind="ExternalOutput"),
        }

    with tile.TileContext(nc) as tc:
        import contextlib
        ctx = contextlib.ExitStack()
        with ctx:
            const = ctx.enter_context(tc.tile_pool(name="const", bufs=1))
            psT = ctx.enter_context(tc.tile_pool(name="psT", bufs=2, space="PSUM"))
            psMM = ctx.enter_context(tc.tile_pool(name="psMM", bufs=3, space="PSUM"))
            psS = ctx.enter_context(tc.tile_pool(name="psS", bufs=1, space="PSUM"))
            psA = ctx.enter_context(tc.tile_pool(name="psA", bufs=2, space="PSUM"))
            gat = ctx.enter_context(tc.tile_pool(name="gat", bufs=3))
            xtp = ctx.enter_context(tc.tile_pool(name="xtp", bufs=2))
            hp = ctx.enter_context(tc.tile_pool(name="hp", bufs=3))
            sm = ctx.enter_context(tc.tile_pool(name="sm", bufs=3))

            def ps_pair(pool, nm):
                return pool.tile([P, 512], F32, name=nm, tag=pool.name)

            def ps_tr(nm):
                # full 2KB so every pool buffer is PSUM-bank aligned
                return psT.tile([P, 1024], BF, name=nm, tag="psT")

            # ---------------- constants ----------------
            ident = const.tile([P, P], F32)
            make_identity(nc, ident[:])
            ident_b = const.tile([P, P], BF)
            nc.vector.tensor_copy(ident_b[:], ident[:])
            ones_row_b = const.tile([1, P], BF)
            nc.vector.memset(ones_row_b[:], 1.0)

            # weights (bf16, direct DMA)
            W1a_t = [const.tile([k1 - k0, HP], BF, name=f"W1a{i}", tag=f"W1a{i}")
                     for i, (k0, k1) in enumerate(EK)]
            for i, (k0, k1) in enumerate(EK):
                nc.sync.dma_start(W1a_t[i][:], W1a_d[k0:k1, :])
            W2a_t = [const.tile([m1 - m0, HP], BF, name=f"W2a{i}", tag=f"W2a{i}")
                     for i, (m0, m1) in enumerate(HM)]
            for i, (m0, m1) in enumerate(HM):
                nc.sync.dma_start(W2a_t[i][:], W2a_d[m0:m1, :])
            W1c_t = [const.tile([k, HP], BF, name=f"W1c{i}", tag=f"W1c{i}")
                     for i, k in enumerate(XCHUNK)]
            for i in range(5):
                nc.sync.dma_start(W1c_t[i][:], W1c_d[i * 128:i * 128 + XCHUNK[i], :])
            W2c_t = [const.tile([m1 - m0, HP], BF, name=f"W2c{i}", tag=f"W2c{i}")
                     for i, (m0, m1) in enumerate(HM)]
            for i, (m0, m1) in enumerate(HM):
                nc.sync.dma_start(W2c_t[i][:], W2c_d[m0:m1, :])
            GK = [(0, 128), (128, 256), (256, 384), (384, 512)]
            W1g_t = [const.tile([k1 - k0, HP], BF, name=f"W1g{i}", tag=f"W1g{i}")
                     for i, (k0, k1) in enumerate(GK)]
            for i, (k0, k1) in enumerate(GK):
                nc.sync.dma_start(W1g_t[i][:], W1g_d[k0:k1, :])
            W2g_t = [const.tile([m1 - m0, 2], BF, name=f"W2g{i}", tag=f"W2g{i}")
                     for i, (m0, m1) in enumerate(HM)]
            for i, (m0, m1) in enumerate(HM):
                nc.sync.dma_start(W2g_t[i][:], W2g_d[m0:m1, :])

            def bias2(d, nm):
                t = [const.tile([m1 - m0, 1], F32, name=f"b{nm}{i}", tag=f"b{nm}{i}")
                     for i, (m0, m1) in enumerate(HM)]
                for i, (m0, m1) in enumerate(HM):
                    nc.sync.dma_start(t[i][:], d[m0:m1, :])
                return t
            b1a_t, b2a_t = bias2(b1a_d, "1a"), bias2(b2a_d, "2a")
            b1c_t, b2c_t = bias2(b1c_d, "1c"), bias2(b2c_d, "2c")
            b1g_t = bias2(b1g_d, "1g")
            b2g_t = const.tile([2, 1], F32)
            nc.sync.dma_start(b2g_t[:], b2g_d[:])

            # masks / lengths
            len_t = []
            len_f = []
            for s, ld in ((0, len1_d), (1, len2_d)):
                lt = const.tile([nb, 1], I32, name=f"len{s}", tag=f"len{s}")
                nc.sync.dma_start(lt[:], ld[:])
                len_t.append(lt)
                lf = const.tile([1, nb], I32, name=f"lenf{s}", tag=f"lenf{s}")
                nc.sync.dma_start(lf[:], ld[:].rearrange("n one -> one n"))
                len_f.append(lf)
            iota_t = const.tile([nb, L], I32)
            nc.gpsimd.iota(iota_t[:], pattern=[[1, L]], base=0, channel_multiplier=0)
            iota2 = const.tile([1, 2 * L], I32)   # 0..255 twice
            nc.gpsimd.iota(iota2[:], pattern=[[0, 2], [1, L]], base=0, channel_multiplier=0)

            lmT = []   # transposed logmask1 [128, nb] f32, 2 chunks (exp bias)
            mTb = []   # transposed 0/1 masks [128, nb] bf16, 2 chunks per sentence
            mTf = []   # same masks in f32 (tensor_scalar scalar2 operand)
            for s in range(2):
                m = const.tile([nb, L], F32, name=f"mask{s}", tag=f"mask{s}")
                nc.vector.tensor_tensor(m[:], iota_t[:], len_t[s][:].to_broadcast([nb, L]), op=ALU.is_lt)
                lts, mts, mfs = [], [], []
                for c in range(2):
                    if s == 0:
                        lm = const.tile([nb, L], F32, name="lm0", tag="lm0")
                        nc.vector.tensor_scalar(lm[:], m[:], 1.0, 30000.0,
                                                op0=ALU.subtract, op1=ALU.mult)
                        tp = ps_pair(psS, "setup_tp")
                        nc.tensor.transpose(tp[:, 0:nb], lm[:, c * P:(c + 1) * P], ident[:nb, :nb])
                        lt = const.tile([P, nb], F32, name=f"lmT{s}{c}", tag=f"lmT{s}{c}")
                        nc.vector.tensor_copy(lt[:], tp[:, 0:nb])
                        lts.append(lt)
                    tp2 = ps_pair(psS, "setup_tp")
                    nc.tensor.transpose(tp2[:, 0:nb], m[:, c * P:(c + 1) * P], ident[:nb, :nb])
                    mt = const.tile([P, nb], BF, name=f"mTb{s}{c}", tag=f"mTb{s}{c}")
                    nc.vector.tensor_copy(mt[:], tp2[:, 0:nb])
                    mts.append(mt)
                    mf = const.tile([P, nb], F32, name=f"mTf{s}{c}", tag=f"mTf{s}{c}")
                    nc.vector.tensor_copy(mf[:], tp2[:, 0:nb])
                    mfs.append(mf)
                lmT.append(lts)
                mTb.append(mts)
                mTf.append(mfs)

            # token indices, transposed to [128, nb] int32 per chunk
            sT = []
            for s, sd in ((0, s1_d), (1, s2_d)):
                st = const.tile([nb, L], I32, name=f"s{s}", tag=f"s{s}")
                nc.sync.dma_start(st[:], sd[:])
                sf = const.tile([nb, L], F32, name=f"sf{s}", tag=f"sf{s}")
                nc.vector.tensor_copy(sf[:], st[:])
                chunks = []
                for c in range(2):
                    tp = ps_pair(psS, "setup_tp")
                    nc.tensor.transpose(tp[:, 0:nb], sf[:, c * P:(c + 1) * P], ident[:nb, :nb])
                    tf = const.tile([P, nb], F32, name=f"sTf{s}{c}", tag=f"sTf{s}{c}")
                    nc.vector.tensor_copy(tf[:], tp[:, 0:nb])
                    ti = const.tile([P, nb], I32, name=f"sTi{s}{c}", tag=f"sTi{s}{c}")
                    nc.vector.tensor_copy(ti[:], tf[:])
                    chunks.append(ti)
                sT.append(chunks)

            # v accumulators [(128|72), nb] per H-chunk per sentence
            v_all = [[const.tile([m1 - m0, nb], F32, name=f"v{s}{m}", tag=f"v{s}{m}")
                      for m, (m0, m1) in enumerate(HM)] for s in range(2)]

            # ---------------- per-batch loop (1-batch software pipeline) ----
            # H1(b): gather -> eT transposes -> attend MLP -> scores -> exp.
            # H2(b): u1 transpose -> attention sums -> normalize -> compare.
            # Emission order H1(b+1) before H2(b) keeps independent PE work
            # between the engine handoffs of the serial softmax chain.
            def emit_h1(b):
                st = {'b': b}
                lens2 = sm.tile([1, 2 * L], I32, name="lens2", tag="lens2")
                for s in range(2):
                    nc.vector.tensor_copy(lens2[:, s * L:(s + 1) * L],
                                          len_f[s][:, b:b + 1].to_broadcast([1, L]))
                mrow2 = sm.tile([1, 2 * L], F32, name="mrow2", tag="mrow2")
                nc.vector.tensor_tensor(mrow2[:], iota2[:], lens2[:], op=ALU.is_lt)
                lmrow2 = sm.tile([1, 2 * L], BF, name="lmrow2", tag="lmrow2")
                nc.vector.tensor_scalar(lmrow2[:], mrow2[:], 1.0, 30000.0,
                                        op0=ALU.subtract, op1=ALU.mult)
                st['lmrow2'] = lmrow2

                eR = [[], []]
                for s in range(2):
                    for c in range(2):
                        er = gat.tile([P, E1], BF, name=f"eR{s}{c}", tag=f"eR{s}{c}")
                        nc.gpsimd.indirect_dma_start(
                            out=er[:, 0:EMBED], out_offset=None, in_=emb_d[:],
                            in_offset=bass.IndirectOffsetOnAxis(ap=sT[s][c][:, b:b + 1], axis=0),
                        )
                        if b < 3:
                            nc.vector.memset(er[:, EMBED:E1], 1.0)
                        eR[s].append(er)
                e2m = []
                for c in range(2):
                    em = gat.tile([P, E1], BF, name=f"e2m{c}", tag=f"e2m{c}")
                    nc.gpsimd.tensor_tensor(em[:], eR[1][c][:],
                                            mTb[1][c][:, b:b + 1].to_broadcast([P, E1]), op=ALU.mult)
                    e2m.append(em)
                st['eR'] = eR
                st['e2m'] = e2m

                XT = xtp.tile([P, 10 * L], BF, name="XT", tag="XT")
                for s in range(2):
                    if b < 2:
                        nc.vector.memset(XT[0:64, 4 * L + s * L:4 * L + (s + 1) * L], 0.0)
                    tp = ps_tr(f"eTtp{s}")
                    for k, (k0, k1) in enumerate(EK):
                        for c in range(2):
                            nc.tensor.transpose(tp[:k1 - k0, k * L + c * P:k * L + (c + 1) * P],
                                                eR[s][c][:, k0:k1], ident_b[:])
                    for k in range(2):
                        nc.any.tensor_copy(XT[:, 2 * k * L + s * L:2 * k * L + (s + 1) * L],
                                           tp[:, k * L:(k + 1) * L])
                    nc.any.tensor_copy(XT[0:44, 4 * L + s * L:4 * L + (s + 1) * L],
                                       tp[0:44, 2 * L:3 * L])
                st['XT'] = XT

                ha = []
                for m, (m0, m1) in enumerate(HM):
                    pa = ps_pair(psMM, "pa")
                    for k, (k0, k1) in enumerate(EK):
                        nc.tensor.matmul(pa[:m1 - m0, :], W1a_t[k][:, m0:m1],
                                         XT[0:k1 - k0, 2 * k * L:2 * (k + 1) * L],
                                         start=(k == 0), stop=(k == 2))
                    h = hp.tile([m1 - m0, 2 * L], BF, name=f"ha{m}", tag=f"ha{m}")
                    nc.scalar.activation(h[:], pa[:m1 - m0, :], ACTF.Relu,
                                         bias=b1a_t[m][:], scale=1.0)
                    ha.append(h)
                hT = []
                for m, (m0, m1) in enumerate(HM):
                    pb = ps_pair(psMM, "pb")
                    for k2 in range(2):
                        nc.tensor.matmul(pb[:m1 - m0, :], W2a_t[k2][:, m0:m1], ha[k2][:],
                                         start=(k2 == 0), stop=(k2 == 1))
                    h = hp.tile([m1 - m0, 2 * L], BF, name=f"hT{m}", tag=f"hT{m}")
                    nc.scalar.activation(h[:], pb[:m1 - m0, :], ACTF.Relu,
                                         bias=b2a_t[m][:], scale=1.0)
                    hT.append(h)
                st['hT'] = hT

                ep = ps_pair(psS, "score")
                for ic in range(2):
                    for m in range(2):
                        nc.tensor.matmul(ep[:, ic * L:(ic + 1) * L],
                                         hT[m][:, ic * P:(ic + 1) * P], hT[m][:, L:2 * L],
                                         start=(m == 0), stop=(m == 1))
                u0 = sm.tile([P, 2 * L], BF, name="u0", tag="u0")
                for ic in range(2):
                    nc.scalar.activation(u0[:, ic * L:(ic + 1) * L], ep[:, ic * L:(ic + 1) * L],
                                         ACTF.Exp, bias=lmT[0][ic][:, b:b + 1], scale=1.0)
                st['u0'] = u0
                return st

            def emit_h2(st):
                b = st['b']
                eR, e2m, XT, u0 = st['eR'], st['e2m'], st['XT'], st['u0']
                tpu = ps_tr("u1tp")
                for jc in range(2):
                    for ic in range(2):
                        nc.tensor.transpose(tpu[:, jc * L + ic * P:jc * L + (ic + 1) * P],
                                            u0[:, ic * L + jc * P:ic * L + (jc + 1) * P], ident_b[:])
                u1 = sm.tile([P, 2 * L], BF, name="u1", tag="u1")
                nc.any.tensor_copy(u1[:], tpu[:, 0:2 * L])

                for d in range(2):
                    u_t = u0 if d == 0 else u1
                    rhs = eR[0] if d == 0 else e2m
                    s = 1 - d
                    tt = ps_tr(f"bt{d}")
                    for t_ in range(2):
                        q = ps_pair(psA, f"q{d}{t_}")
                        for c in range(2):
                            nc.tensor.matmul(q[:, 0:E1],
                                             u_t[:, c * L + t_ * P:c * L + (t_ + 1) * P],
                                             rhs[c][:, 0:E1], start=(c == 0), stop=(c == 1))
                        dcol = sm.tile([P, 1], F32, name=f"dc{d}{t_}", tag=f"dc{d}{t_}")
                        nc.vector.tensor_scalar(dcol[:], q[:, EMBED:E1], 1e-20, None, op0=ALU.add)
                        rcp = sm.tile([P, 1], F32, name=f"rcp{d}{t_}", tag=f"rcp{d}{t_}")
                        nc.vector.reciprocal(rcp[:], dcol[:])
                        nrm = sm.tile([P, EMBED], BF, name=f"nrm{d}{t_}", tag=f"nrm{d}{t_}")
                        nc.vector.tensor_scalar(nrm[:], q[:, 0:EMBED], rcp[:],
                                                mTf[1 - d][t_][:, b:b + 1],
                                                op0=ALU.mult, op1=ALU.mult)
                        for t, (c0, c1, poff) in enumerate(BCHUNK):
                            nc.tensor.transpose(
                                tt[poff:poff + (c1 - c0), t * L + t_ * P:t * L + (t_ + 1) * P],
                                nrm[:, c0:c1], ident_b[:])
                    nc.any.tensor_copy(XT[64:128, 4 * L + s * L:4 * L + (s + 1) * L],
                                       tt[64:128, 0:L])
                    nc.any.tensor_copy(XT[:, 6 * L + s * L:6 * L + (s + 1) * L],
                                       tt[:, L:2 * L])
                    nc.any.tensor_copy(XT[0:108, 8 * L + s * L:8 * L + (s + 1) * L],
                                       tt[0:108, 2 * L:3 * L])

                r1 = []
                for m, (m0, m1) in enumerate(HM):
                    pc = ps_pair(psMM, "pc")
                    for k in range(5):
                        nc.tensor.matmul(pc[:m1 - m0, :], W1c_t[k][:, m0:m1],
                                         XT[0:XCHUNK[k], 2 * k * L:2 * (k + 1) * L],
                                         start=(k == 0), stop=(k == 4))
                    r = hp.tile([m1 - m0, 2 * L], BF, name=f"r1{m}", tag=f"r1{m}")
                    nc.scalar.activation(r[:], pc[:m1 - m0, :], ACTF.Relu,
                                         bias=b1c_t[m][:], scale=1.0)
                    r1.append(r)
                for m, (m0, m1) in enumerate(HM):
                    pd = ps_pair(psMM, "pd")
                    for k2 in range(2):
                        nc.tensor.matmul(pd[:m1 - m0, :], W2c_t[k2][:, m0:m1], r1[k2][:],
                                         start=(k2 == 0), stop=False)
                    nc.tensor.matmul(pd[:m1 - m0, :], ones_row_b[:, 0:m1 - m0],
                                     st['lmrow2'][:], start=False, stop=True)
                    for s in range(2):
                        scr = hp.tile([m1 - m0, L], BF, name=f"scr{s}{m}", tag=f"scr{s}{m}")
                        nc.scalar.activation(scr[:], pd[:m1 - m0, s * L:(s + 1) * L],
                                             ACTF.Relu, bias=b2c_t[m][:],
                                             scale=1.0, accum_out=v_all[s][m][:, b:b + 1])

                if dbg and b == 0:
                    nc.sync.dma_start(dbg_d['XT'][:], XT[:])
                    nc.sync.dma_start(dbg_d['u0'][:], u0[:])
                    nc.sync.dma_start(dbg_d['u1'][:], u1[:])
                    nc.sync.dma_start(dbg_d['hT0'][:], st['hT'][0][:])

            prev = emit_h1(0)
            for b in range(1, nb):
                cur = emit_h1(b)
                emit_h2(prev)
                prev = cur
            emit_h2(prev)

            # ---------------- aggregate ----------------
            if dbg:
                nc.sync.dma_start(dbg_d['v00'][:], v_all[0][0][:])
            vb = []
            for s in range(2):
                for m, (m0, m1) in enumerate(HM):
                    t = const.tile([m1 - m0, nb], BF, name=f"vb{s}{m}", tag=f"vb{s}{m}")
                    nc.vector.tensor_copy(t[:], v_all[s][m][:])
                    vb.append(t)
            g1 = []
            gp = ps_pair(psMM, "pa")
            for m, (m0, m1) in enumerate(HM):
                for k in range(4):
                    nc.tensor.matmul(gp[:m1 - m0, m * nb:(m + 1) * nb],
                                     W1g_t[k][:, m0:m1], vb[k][:],
                                     start=(k == 0), stop=(k == 3))
            for m, (m0, m1) in enumerate(HM):
                g = const.tile([m1 - m0, nb], BF, name=f"g1{m}", tag=f"g1{m}")
                nc.scalar.activation(g[:], gp[:m1 - m0, m * nb:(m + 1) * nb],
                                     ACTF.Relu, bias=b1g_t[m][:], scale=1.0)
                g1.append(g)
            op = ps_pair(psMM, "pb")
            for k2 in range(2):
                nc.tensor.matmul(op[0:2, 0:nb], W2g_t[k2][:], g1[k2][:],
                                 start=(k2 == 0), stop=(k2 == 1))
            osb = const.tile([2, nb], F32, name="osb", tag="osb")
            nc.scalar.activation(osb[:], op[0:2, 0:nb], ACTF.Identity, bias=b2g_t[:], scale=1.0)
            nc.sync.dma_start(out_d[:].rearrange("b o -> o b"), osb[:])

    nc.compile()
    return nc


def _shard_inputs(inputs, nb=BC, ncores=NCORES):
    import ml_dtypes
    bf16 = ml_dtypes.bfloat16
    f = np.ascontiguousarray

    emb_bf = f(inputs['emb'].astype(bf16))
    # Hidden dim zero-padded 200 -> 256 so every weight chunk is a full 128
    # columns (enables the PE fast weight load).  W1c additionally packed into
    # 5 chunks of 128 rows: [0:256] e-rows, chunk2 = 44 e-tail rows + 20 zero
    # rows + 64 beta rows, then beta rows 64:192, 192:300.
    HPad = 256
    W1c = inputs['W1c'].astype(np.float32)
    W1c_p = np.zeros((640, HPad), np.float32)
    W1c_p[0:256, 0:HIDDEN] = W1c[0:256]
    W1c_p[256:300, 0:HIDDEN] = W1c[256:300]
    W1c_p[320:384, 0:HIDDEN] = W1c[300:364]
    W1c_p[384:512, 0:HIDDEN] = W1c[364:492]
    W1c_p[512:620, 0:HIDDEN] = W1c[492:600]
    W1a_p = np.zeros((EMBED, HPad), np.float32)
    W1a_p[:, 0:HIDDEN] = inputs['W1a']
    W2a_p = np.zeros((HPad, HPad), np.float32)
    W2a_p[0:HIDDEN, 0:HIDDEN] = inputs['W2a']
    W2c_p = np.zeros((HPad, HPad), np.float32)
    W2c_p[0:HIDDEN, 0:HIDDEN] = inputs['W2c']
    # v layout is [s0m0 (128) | s0m1 (72+56 pad) | s1m0 | s1m1]
    W1g = inputs['W1g'].astype(np.float32)
    W1g_p = np.zeros((512, HPad), np.float32)
    W1g_p[0:128, 0:HIDDEN] = W1g[0:128]
    W1g_p[128:200, 0:HIDDEN] = W1g[128:200]
    W1g_p[256:384, 0:HIDDEN] = W1g[200:328]
    W1g_p[384:456, 0:HIDDEN] = W1g[328:400]
    W2g_p = np.zeros((HPad, 2), np.float32)
    W2g_p[0:HIDDEN] = inputs['W2g']
    wb = {'W1a': f(W1a_p.astype(bf16)), 'W2a': f(W2a_p.astype(bf16)),
          'W1c': f(W1c_p.astype(bf16)), 'W2c': f(W2c_p.astype(bf16)),
          'W1g': f(W1g_p.astype(bf16)), 'W2g': f(W2g_p.astype(bf16))}

    def bpad(x):
        p = np.zeros((256, 1), np.float32)
        p[0:HIDDEN, 0] = np.asarray(x).ravel()
        return p

    maps = []
    for c in range(ncores):
        sl = slice(c * nb, (c + 1) * nb)
        maps.append(dict(
            emb=emb_bf,
            s1=f(inputs['s1'][sl].astype(np.int32)),
            s2=f(inputs['s2'][sl].astype(np.int32)),
            len1=f(inputs['len1'][sl].reshape(nb, 1).astype(np.int32)),
            len2=f(inputs['len2'][sl].reshape(nb, 1).astype(np.int32)),
            b1a=bpad(inputs['b1a']), b2a=bpad(inputs['b2a']),
            b1c=bpad(inputs['b1c']), b2c=bpad(inputs['b2c']),
            b1g=bpad(inputs['b1g']),
            b2g=f(inputs['b2g'].reshape(-1, 1).astype(np.float32)),
            **wb,
        ))
    return maps


def kernel(**inputs):
    from concourse.bass_utils import run_bass_kernel_spmd
    if 'prog' not in _prog_cache:
        _prog_cache['prog'] = build_program(BC)
    nc = _prog_cache['prog']
    in_maps = _shard_inputs(inputs)
    res = run_bass_kernel_spmd(nc, in_maps, core_ids=list(range(NCORES)))
    out = np.concatenate([res.results[c]["out"] for c in range(NCORES)], axis=0)
    return out.astype(np.float32)
